# revision 25
# baseline (speedup 1.0000x reference)
"""Trainium2 Bass kernel for windowed Conv1d(k=3) + sigmoid gating.

Reference (B=16, T=960, D=1024, W=10): windows of 10 conv'd independently
with per-window zero pad 1:
    cnn[t, d] = sum_{k,c} conv_w[d, c, k] * xpad[t + k, c]
    out = cnn * sigmoid(cnn @ gate_w.T + gate_b)

Strategy: data parallel over 8 cores (2 batches / 192 windows / core).
Hybrid Winograd conv: two F(4,3) tiles (outputs 0-3 from xp[0:6], outputs
4-7 from xp[4:10]) at points {0,1,-1,2,-1/2,inf} plus an F(2,3) tail
(outputs 8,9 from xp[8:12]) at points {0,1,-1,inf} folded into the same
weight streams => 16 muls/window (vs 30 direct).  Streams use canonical
Vandermonde weights g_b = [1,b,b^2].W (normalization in the host-side
V^{-T} input transforms).

Per core: 2 groups of 96 windows (group == local batch row).  Per
(group, dck): 6 accumulation chains into one 4-bank PSUM slot packed
{m1+m3 | m2+m4 | m0 | m5} (chains may not cross banks).  The combine is
copy-first: ACT copies each bank to bf16 SBUF as its chains finish
(frees the slot fast => no PE stall at unit boundaries), then the A^T
combine runs in bf16 on DVE/GpSimd/ACT with ~1.6us/unit slack per engine.
Tails use a single class (no rotation): stream columns are [A|B|T].

Phases: g0 convs -> all g0 gates -> g1 convs -> g1 gates; xt[g1]
reloads into the xt slots during the g0 gate phase (25us window);
gwr is ordered after cwr in the DMA stream (first-use order).  A short
cold warmup (~12 matmuls) covers the preamble->first-data window.
"""

import numpy as np
import ml_dtypes

import concourse.bacc as bacc
import concourse.bass as bass
import concourse.tile as tile
from concourse import mybir
from concourse.bass_utils import run_bass_kernel_spmd

BF16 = ml_dtypes.bfloat16

B, T, D, W = 16, 960, 1024, 10
NCORES = 8
BC = B // NCORES             # batches per core (2)
NWIN = BC * T // W           # windows per core (192)
RC = NWIN * W                # output rows per core (1920)
NG = 2                       # groups per core (group == local batch row)
GWN = NWIN // NG             # windows per group (96)
GN = GWN * W                 # output cols per group (960)
NCH = D // 128               # channel chunks (8)
AF = mybir.ActivationFunctionType

INF = "inf"
PTS = [0.0, 1.0, -1.0, 2.0, -0.5, INF]          # logical nodes
TPTS = [0.0, 1.0, -1.0, INF]                     # tail nodes
# storage (chain) order of streams: nodes per storage slot
SNODE = [1.0, 2.0, -1.0, -0.5, 0.0, INF]         # m1 m3 m2 m4 m0 m5
SW = [3, 2, 3, 2, 3, 3]                          # width in 96-col blocks
WID = [w * GWN for w in SW]                      # [288,192,288,192,288,288]
# tail row in TPTS for storage streams that carry a tail block
TROW = {0: 1, 2: 2, 4: 0, 5: 3}
# per-half storage streams (matches xt half layout and cwr layout)
HSTREAMS = [(0, 1, 2), (3, 4, 5)]
HCOLS = 6144                                     # cols per xt half (768*8)
# psum col offset per storage stream (4 banks of 512 fp32)
POFF = [0, 288, 512, 800, 1024, 1536]
# bf16 mS col offset per storage stream (packed, 1536 cols)
MOFF = [0, 288, 480, 768, 960, 1248]
# cwr col offset of (storage stream, ck) block
SOFF_X = []                                      # xt col offset per stream
_off = 0
for _s in range(6):
    SOFF_X.append(_off if _s < 3 else _off - HCOLS)
    _off += WID[_s] * NCH
AB = 2 * GWN                                     # 192 (A|B block)
TL, TH = AB, AB + GWN                            # tail block 192:288


def _vinv_T(points):
    n = len(points)
    V = np.zeros((n, n))
    for j, b in enumerate(points):
        if b is INF:
            V[j, n - 1] = 1.0
        else:
            V[j] = [float(b) ** i for i in range(n)]
    return np.linalg.inv(V).T


BA = _vinv_T(PTS)
BT4 = _vinv_T(TPTS)
LOGI = [PTS.index(b) for b in SNODE]             # storage -> logical row


def _build():
    nc = bacc.Bacc("TRN2", target_bir_lowering=False, debug=False)

    # xt[g*2+h]: [cc, (storage stream, ck, col)] transformed input halves
    xt = nc.dram_tensor("xt", [NG * 2, 128, HCOLS], mybir.dt.bfloat16,
                        kind="ExternalInput")
    # cwr[dck]: [cc, (s*8+ck)*128+dd] conv lhsT blocks, storage order
    cwr = nc.dram_tensor("cwr", [NCH, 128, 6 * NCH * 128], mybir.dt.bfloat16,
                         kind="ExternalInput")
    # gwr[eck]: [dd, dck*128+ee] gate lhsT blocks
    gwr = nc.dram_tensor("gwr", [NCH, 128, NCH * 128], mybir.dt.bfloat16,
                         kind="ExternalInput")
    cb = nc.dram_tensor("cb", [128, NCH], mybir.dt.float32, kind="ExternalInput")
    gb = nc.dram_tensor("gb", [128, NCH], mybir.dt.float32, kind="ExternalInput")
    outT = nc.dram_tensor("outT", [D, RC], mybir.dt.float32, kind="ExternalOutput")

    with tile.TileContext(nc) as tc:
        with (
            tc.tile_pool(name="consts", bufs=1) as consts,
            tc.tile_pool(name="xtp", bufs=1) as xtp,
            tc.tile_pool(name="cnn", bufs=1) as cnnp,
            tc.tile_pool(name="ms", bufs=2) as msp,
            tc.tile_pool(name="work", bufs=2) as work,
            tc.tile_pool(name="gout", bufs=4) as gout,
            tc.tile_pool(name="psum", bufs=2, space="PSUM") as psum,
        ):
            cwr_sb = [None] * NCH
            xt_sb = [None, None]

            def load_xt_half(g, h):
                # scalar (ACT) HWDGE queue: parallel to the sync queue
                t = xtp.tile([128, HCOLS], mybir.dt.bfloat16, tag=f"xt{h}",
                             name=f"xt{h}_g{g}")
                nc.scalar.dma_start(t[:], xt[g * 2 + h])
                xt_sb[h] = t

            # ---- DMA ramp (first-use order, per-stream pieces for dck0) ----
            cw0 = consts.tile([128, 6 * NCH * 128], mybir.dt.bfloat16, tag="cw0")
            cwr_sb[0] = cw0
            xt_sb[0] = xtp.tile([128, HCOLS], mybir.dt.bfloat16, tag="xt0",
                                name="xt0_g0")
            xt_sb[1] = xtp.tile([128, HCOLS], mybir.dt.bfloat16, tag="xt1",
                                name="xt1_g0")
            # dual-queue ramp: cw0 stream pieces + cwr1-7 + gwr on the sync
            # (SP) queue; xt pieces + cb/gb on the scalar (ACT) queue.  Each
            # DMA instruction costs ~650ns of queue issue time, so splitting
            # halves the serial issue latency of the ramp.
            SB = NCH * 128                       # cwr cols per stream (1024)
            for s in range(6):
                h = 0 if s < 3 else 1
                o = SOFF_X[s]
                nparts = 2 if s == 0 else 1      # halve the very first pieces
                cwn, xtn = SB // nparts, NCH * WID[s] // nparts
                for p in range(nparts):
                    nc.sync.dma_start(
                        cw0[:, s * SB + p * cwn:s * SB + (p + 1) * cwn],
                        cwr[0][:, s * SB + p * cwn:s * SB + (p + 1) * cwn])
                    nc.sync.dma_start(
                        xt_sb[h][:, o + p * xtn:o + (p + 1) * xtn],
                        xt[h][:, o + p * xtn:o + (p + 1) * xtn])
            # tiny bias loads on the scalar queue (off the critical sync FIFO)
            cb_sb = consts.tile([128, NCH], mybir.dt.float32, tag="cb")
            nc.scalar.dma_start(cb_sb[:], cb[:])
            gb_sb = consts.tile([128, NCH], mybir.dt.float32, tag="gb")
            nc.scalar.dma_start(gb_sb[:], gb[:])
            for dck in range(1, NCH):
                cwt = consts.tile([128, 6 * NCH * 128], mybir.dt.bfloat16,
                                  tag=f"cw{dck}")
                cwr_sb[dck] = cwt
                nc.sync.dma_start(cwt[:], cwr[dck])
            gwr_sb = []
            for eck in range(NCH):
                t = consts.tile([128, NCH * 128], mybir.dt.bfloat16, tag=f"gw{eck}")
                nc.sync.dma_start(t[:], gwr[eck])
                gwr_sb.append(t)

            # ---- HAM warmup over the preamble->first-data window ----
            scr = consts.tile([128, 256], mybir.dt.bfloat16, tag="scr")
            nc.gpsimd.memset(scr[:], 0.0)
            # enough cold matmuls (~5.5us) to bridge clock-start -> first
            # conv data; keeps the HAM busy-window unbroken so real chains
            # run at 8/8 from the start
            wps = psum.tile([128, 2048], mybir.dt.float32, tag="cps")
            for _ in range(14):
                nc.tensor.matmul(wps[:, :256], scr[:, :128], scr[:, :256],
                                 start=True, stop=True)

            cnn_t = [[None] * NCH for _ in range(NG)]

            def conv_unit(g, dck, pad_intra=False, pad_end=0):
                ps = psum.tile([128, 2048], mybir.dt.float32, tag="cps")
                for s in range(6):
                    h = 0 if s < 3 else 1
                    w = WID[s]
                    for ck in range(NCH):
                        nc.tensor.matmul(
                            ps[:, POFF[s]:POFF[s] + w],
                            cwr_sb[dck][:, (s * NCH + ck) * 128:
                                        (s * NCH + ck + 1) * 128],
                            xt_sb[h][:, SOFF_X[s] + ck * w:
                                     SOFF_X[s] + (ck + 1) * w],
                            start=(ck == 0),
                            stop=(ck == NCH - 1),
                        )
                    # anti-idle fillers into this slot's unused pad cols
                    # (bank3 tail): keep HAM at 8/8 while the ramp DMA
                    # trickles chain inputs in; never read.
                    npad = (3 if pad_intra else 0) if s < 5 else pad_end
                    for _ in range(npad):
                        nc.tensor.matmul(ps[:, 1824:2048], scr[:, :128],
                                         scr[:, :224], start=True, stop=True)
                combine(g, dck, ps)

            def combine(g, dck, ps):
                cbs = cb_sb[:, dck:dck + 1]
                bf = mybir.dt.bfloat16
                mS = msp.tile([128, 1536], bf, tag="ms")

                # copy-first: one ACT copy per psum bank, in chain order;
                # the slot frees after c4 (~1us after the unit's last MM)
                nc.scalar.activation(mS[:, 0:480], ps[:, 0:480], AF.Copy)
                nc.scalar.activation(mS[:, 480:960], ps[:, 512:992], AF.Copy)
                nc.scalar.activation(mS[:, 960:1248], ps[:, 1024:1312], AF.Copy)
                nc.scalar.activation(mS[:, 1248:1536], ps[:, 1536:1824], AF.Copy)
                m1 = mS[:, MOFF[0]:MOFF[0] + 288]
                m3 = mS[:, MOFF[1]:MOFF[1] + 192]
                m2 = mS[:, MOFF[2]:MOFF[2] + 288]
                m4 = mS[:, MOFF[3]:MOFF[3] + 192]
                m0 = mS[:, MOFF[4]:MOFF[4] + 288]
                m5 = mS[:, MOFF[5]:MOFF[5] + 288]

                def st(tag, n):
                    return work.tile([128, n], bf, tag=tag, name=tag)

                cnn = cnnp.tile([128, GN], bf, tag=f"cnn{g}_{dck}")
                cnn_t[g][dck] = cnn
                v = cnn[:].rearrange("p (t w) -> p t w", w=GWN)

                def outab(t0):
                    # A-tile output t0 and B-tile output t0+4: [128, 2, 96]
                    return v[:, t0:t0 + 5:4]

                # u_k = b^k m3 + cb on ACT; h_k = (-1/2)^k m4 on DVE
                u1 = st("u1", AB)
                nc.scalar.activation(u1[:], m3, AF.Identity, bias=cbs, scale=2.0)
                u2 = st("u2", AB)
                nc.scalar.activation(u2[:], m3, AF.Identity, bias=cbs, scale=4.0)
                u3 = st("u3", AB)
                nc.scalar.activation(u3[:], m3, AF.Identity, bias=cbs, scale=8.0)
                s1 = st("s1", 288); nc.vector.tensor_add(s1[:], m1, m2)
                d1 = st("d1", 288); nc.vector.tensor_sub(d1[:], m1, m2)
                h1 = st("h1", AB); nc.vector.tensor_scalar_mul(h1[:], m4, -0.5)
                h2 = st("h2", AB); nc.vector.tensor_scalar_mul(h2[:], m4, 0.25)
                h3 = st("h3", AB); nc.vector.tensor_scalar_mul(h3[:], m4, -0.125)
                a0 = st("a0", AB); nc.vector.tensor_add(a0[:], m3, m4)
                t0 = st("t0", AB); nc.vector.tensor_scalar_add(t0[:], a0[:], cbs)
                P = st("P", 288)
                nc.gpsimd.tensor_add(P[:], m0, s1[:])
                nc.gpsimd.tensor_add(outab(0), P[:, :AB], t0[:])          # y0
                w1 = st("w1", AB); nc.gpsimd.tensor_add(w1[:], d1[:, :AB], u1[:])
                nc.gpsimd.tensor_add(outab(1), w1[:], h1[:])              # y1
                w2 = st("w2", AB); nc.gpsimd.tensor_add(w2[:], s1[:, :AB], u2[:])
                nc.gpsimd.tensor_add(outab(2), w2[:], h2[:])              # y2
                w3 = st("w3", AB); nc.vector.tensor_add(w3[:], d1[:, :AB], u3[:])
                x3 = st("x3", AB); nc.vector.tensor_add(x3[:], w3[:], h3[:])
                nc.gpsimd.tensor_add(outab(3), x3[:], m5[:, :AB])         # y3
                # tails: y8 = P[T]+cb ; y9 = d1[T] + m5[T] + cb
                nc.vector.tensor_scalar_add(v[:, 8, :], P[:, TL:TH], cbs)
                e9 = st("e9", GWN)
                nc.vector.tensor_add(e9[:], d1[:, TL:TH], m5[:, TL:TH])
                nc.vector.tensor_scalar_add(v[:, 9, :], e9[:], cbs)

            def gate_quad(g, eck0, tail=False):
                # 4 chains (eck0,c0),(eck0,c1),(eck0+1,c0),(eck0+1,c1) into
                # banks 0-3 of one psum slot, accumulated dck-MAJOR: the
                # cnn[dck=7]-dependent matmuls land ~5.7us of PE work after
                # a conv phase end (covers the trailing combine backlog),
                # and the deep chain pipeline avoids psum-WAR stalls.
                # tail=True (very last quad): chain 3 runs alone after the
                # other three so only its short split epilog trails the
                # kernel's last matmul.
                chains = [(eck0, 0), (eck0, 1), (eck0 + 1, 0), (eck0 + 1, 1)]

                def epilog(pst, q, eck, c, split=False):
                    gt = gout.tile([128, 480], mybir.dt.bfloat16, tag="gate")
                    ot = gout.tile([128, 480], mybir.dt.float32, tag="out")
                    chunks = ((0, 240), (240, 480)) if split else ((0, 480),)
                    for lo, hi in chunks:
                        nc.scalar.activation(gt[:, lo:hi],
                                             pst[:, q * 512 + lo:q * 512 + hi],
                                             AF.Sigmoid,
                                             bias=gb_sb[:, eck:eck + 1])
                        nc.vector.tensor_mul(ot[:, lo:hi],
                                             cnn_t[g][eck][:, c * 480 + lo:
                                                           c * 480 + hi],
                                             gt[:, lo:hi])
                        # final pieces go out via the gpsimd (SWDGE) queue,
                        # overlapping the sync queue's issue latency
                        eng = nc.gpsimd if split else nc.sync
                        eng.dma_start(
                            outT[eck * 128:(eck + 1) * 128,
                                 g * GN + c * 480 + lo:g * GN + c * 480 + hi],
                            ot[:, lo:hi])

                if tail:
                    # chains 0-2 dck-major in ps2; chain 3 in its OWN
                    # rotated tile with matmuls emitted BEFORE the other
                    # epilogs (tile-granular WAR tracking would stall them
                    # on the sigmoids reading ps2), so only chain 3's short
                    # split epilog trails the kernel's final matmul.
                    ps2 = psum.tile([128, 2048], mybir.dt.float32, tag="cps")
                    for dck in range(NCH):
                        for q in range(3):
                            eck, c = chains[q]
                            nc.tensor.matmul(
                                ps2[:, q * 512:q * 512 + 480],
                                gwr_sb[eck][:, dck * 128:(dck + 1) * 128],
                                cnn_t[g][dck][:, c * 480:(c + 1) * 480],
                                start=(dck == 0),
                                stop=(dck == NCH - 1),
                            )
                    ps3 = psum.tile([128, 2048], mybir.dt.float32, tag="cps",
                                    name="tailq3")
                    eck3, c3 = chains[3]
                    for dck in range(NCH):
                        nc.tensor.matmul(
                            ps3[:, :480],
                            gwr_sb[eck3][:, dck * 128:(dck + 1) * 128],
                            cnn_t[g][dck][:, c3 * 480:(c3 + 1) * 480],
                            start=(dck == 0),
                            stop=(dck == NCH - 1),
                        )
                    for q in range(3):
                        epilog(ps2, q, *chains[q])
                    epilog(ps3, 0, eck3, c3, split=True)
                else:
                    ps2 = psum.tile([128, 2048], mybir.dt.float32, tag="cps")
                    for dck in range(NCH):
                        for q, (eck, c) in enumerate(chains):
                            nc.tensor.matmul(
                                ps2[:, q * 512:q * 512 + 480],
                                gwr_sb[eck][:, dck * 128:(dck + 1) * 128],
                                cnn_t[g][dck][:, c * 480:(c + 1) * 480],
                                start=(dck == 0),
                                stop=(dck == NCH - 1),
                            )
                    for q, (eck, c) in enumerate(chains):
                        epilog(ps2, q, eck, c)

            # phase g0 convs (cwr streams in behind; early units pad their
            # DMA-trickle idle with anti-idle matmuls)
            for dck in range(NCH):
                conv_unit(0, dck, pad_intra=(dck < 2),
                          pad_end=(4 if dck < 4 else 0))
            # xt g1 reloads during the g0 gate phase (WAR on the xt slots
            # releases at g0-conv end; the 25us gate window covers it)
            load_xt_half(1, 0)
            load_xt_half(1, 1)
            for eck0 in range(0, NCH, 2):
                gate_quad(0, eck0)
            for dck in range(NCH):
                conv_unit(1, dck)
            for eck0 in range(0, NCH, 2):
                gate_quad(1, eck0, tail=(eck0 == NCH - 2))
    nc.compile()
    return nc


def _prep_weights(conv_w, conv_b, gate_w, gate_b):
    W0, W1, W2 = [conv_w[:, :, k].astype(np.float64) for k in range(3)]
    g = []
    for b in SNODE:
        g.append(W2 if b is INF else W0 + b * W1 + b * b * W2)
    garr = np.stack(g)                                   # [6 storage, Do, Di]
    gv = garr.reshape(6, NCH, 128, NCH, 128)             # [s, dck, dd, ck, cc]
    cw_host = np.ascontiguousarray(gv.transpose(1, 4, 0, 3, 2)).reshape(
        NCH, 128, 6 * NCH * 128).astype(BF16)
    gwt = gate_w.T.reshape(NCH, 128, NCH, 128)           # [dck, dd, eck, ee]
    gw_host = np.ascontiguousarray(gwt.transpose(2, 1, 0, 3)).reshape(
        NCH, 128, NCH * 128).astype(BF16)
    cb_host = np.ascontiguousarray(conv_b.reshape(NCH, 128).T).astype(np.float32)
    gb_host = np.ascontiguousarray(gate_b.reshape(NCH, 128).T).astype(np.float32)
    return cw_host, gw_host, cb_host, gb_host


def _prep_core_x(x_shard):
    # x_shard [BC, T, D] -> xt [NG*2, 128, HCOLS]
    xw = x_shard.reshape(NWIN, W, D).astype(np.float64)
    xp = np.pad(xw, ((0, 0), (1, 1), (0, 0)))            # [192, 12, D]
    xt_host = np.empty((NG * 2, 128, HCOLS), BF16)
    for g in range(NG):
        ws = xp[g * GWN:(g + 1) * GWN]                   # [96, 12, D]
        xA = np.einsum('ji,wic->jwc', BA, ws[:, 0:6])    # [6, 96, D]
        xB = np.einsum('ji,wic->jwc', BA, ws[:, 4:10])
        xT = np.einsum('ji,wic->jwc', BT4, ws[:, 8:12])  # [4, 96, D]
        for s in range(6):
            j = LOGI[s]
            parts = [xA[j], xB[j]]
            if s in TROW:
                parts.append(xT[TROW[s]])
            S = np.concatenate(parts, axis=0)            # [WID[s], D]
            blk = S.T.reshape(NCH, 128, WID[s]).transpose(1, 0, 2)
            h = 0 if s < 3 else 1
            o = SOFF_X[s]
            xt_host[g * 2 + h, :, o:o + NCH * WID[s]] = \
                blk.reshape(128, NCH * WID[s]).astype(BF16)
    return xt_host


def _unshard_core(o):
    # o: [D, RC] cols ordered (g, t, w); group g == local batch row,
    # window w at in-window position t -> time w*10+t
    return np.ascontiguousarray(
        o.reshape(D, NG, W, GWN).transpose(1, 3, 2, 0).reshape(BC, T, D))


_NC_CACHE = None


def _prep_in_maps(x, conv_w, conv_b, gate_w, gate_b):
    cw_host, gw_host, cb_host, gb_host = _prep_weights(
        conv_w, conv_b, gate_w, gate_b)
    return [
        {"xt": _prep_core_x(x[BC * i:BC * (i + 1)]), "cwr": cw_host,
         "gwr": gw_host, "cb": cb_host, "gb": gb_host}
        for i in range(NCORES)
    ]


def kernel(x, conv_w, conv_b, gate_w, gate_b):
    global _NC_CACHE
    x = np.asarray(x, np.float32)
    conv_w = np.asarray(conv_w, np.float32)
    conv_b = np.asarray(conv_b, np.float32)
    gate_w = np.asarray(gate_w, np.float32)
    gate_b = np.asarray(gate_b, np.float32)

    in_maps = _prep_in_maps(x, conv_w, conv_b, gate_w, gate_b)
    if _NC_CACHE is None:
        _NC_CACHE = _build()
    res = run_bass_kernel_spmd(_NC_CACHE, in_maps,
                               core_ids=list(range(NCORES))).results

    out = np.empty((B, T, D), np.float32)
    for i in range(NCORES):
        out[BC * i:BC * (i + 1)] = _unshard_core(np.asarray(res[i]["outT"]))
    return out


# revision 26
# speedup vs baseline: 1.0069x; 1.0069x over previous
"""Trainium2 Bass kernel for windowed Conv1d(k=3) + sigmoid gating.

Reference (B=16, T=960, D=1024, W=10): windows of 10 conv'd independently
with per-window zero pad 1:
    cnn[t, d] = sum_{k,c} conv_w[d, c, k] * xpad[t + k, c]
    out = cnn * sigmoid(cnn @ gate_w.T + gate_b)

Strategy: data parallel over 8 cores (2 batches / 192 windows / core).
Hybrid Winograd conv: two F(4,3) tiles (outputs 0-3 from xp[0:6], outputs
4-7 from xp[4:10]) at points {0,1,-1,2,-1/2,inf} plus an F(2,3) tail
(outputs 8,9 from xp[8:12]) at points {0,1,-1,inf} folded into the same
weight streams => 16 muls/window (vs 30 direct).  Streams use canonical
Vandermonde weights g_b = [1,b,b^2].W (normalization in the host-side
V^{-T} input transforms).

Per core: 2 groups of 96 windows (group == local batch row).  Per
(group, dck): 6 accumulation chains into one 4-bank PSUM slot packed
{m1+m3 | m2+m4 | m0 | m5} (chains may not cross banks).  The combine is
copy-first: ACT copies each bank to bf16 SBUF as its chains finish
(frees the slot fast => no PE stall at unit boundaries), then the A^T
combine runs in bf16 on DVE/GpSimd/ACT with ~1.6us/unit slack per engine.
Tails use a single class (no rotation): stream columns are [A|B|T].

Phases: g0 convs -> all g0 gates -> g1 convs -> g1 gates; xt[g1]
reloads into the xt slots during the g0 gate phase (25us window);
gwr is ordered after cwr in the DMA stream (first-use order).  A short
cold warmup (~12 matmuls) covers the preamble->first-data window.
"""

import numpy as np
import ml_dtypes

import concourse.bacc as bacc
import concourse.bass as bass
import concourse.tile as tile
from concourse import mybir
from concourse.bass_utils import run_bass_kernel_spmd

BF16 = ml_dtypes.bfloat16

B, T, D, W = 16, 960, 1024, 10
NCORES = 8
BC = B // NCORES             # batches per core (2)
NWIN = BC * T // W           # windows per core (192)
RC = NWIN * W                # output rows per core (1920)
NG = 2                       # groups per core (group == local batch row)
GWN = NWIN // NG             # windows per group (96)
GN = GWN * W                 # output cols per group (960)
NCH = D // 128               # channel chunks (8)
AF = mybir.ActivationFunctionType

INF = "inf"
PTS = [0.0, 1.0, -1.0, 2.0, -0.5, INF]          # logical nodes
TPTS = [0.0, 1.0, -1.0, INF]                     # tail nodes
# storage (chain) order of streams: nodes per storage slot
SNODE = [1.0, 2.0, -1.0, -0.5, 0.0, INF]         # m1 m3 m2 m4 m0 m5
SW = [3, 2, 3, 2, 3, 3]                          # width in 96-col blocks
WID = [w * GWN for w in SW]                      # [288,192,288,192,288,288]
# tail row in TPTS for storage streams that carry a tail block
TROW = {0: 1, 2: 2, 4: 0, 5: 3}
# per-half storage streams (matches xt half layout and cwr layout)
HSTREAMS = [(0, 1, 2), (3, 4, 5)]
HCOLS = 6144                                     # cols per xt half (768*8)
# psum col offset per storage stream (4 banks of 512 fp32)
POFF = [0, 288, 512, 800, 1024, 1536]
# bf16 mS col offset per storage stream (packed, 1536 cols)
MOFF = [0, 288, 480, 768, 960, 1248]
# cwr col offset of (storage stream, ck) block
SOFF_X = []                                      # xt col offset per stream
_off = 0
for _s in range(6):
    SOFF_X.append(_off if _s < 3 else _off - HCOLS)
    _off += WID[_s] * NCH
AB = 2 * GWN                                     # 192 (A|B block)
TL, TH = AB, AB + GWN                            # tail block 192:288


def _vinv_T(points):
    n = len(points)
    V = np.zeros((n, n))
    for j, b in enumerate(points):
        if b is INF:
            V[j, n - 1] = 1.0
        else:
            V[j] = [float(b) ** i for i in range(n)]
    return np.linalg.inv(V).T


BA = _vinv_T(PTS)
BT4 = _vinv_T(TPTS)
LOGI = [PTS.index(b) for b in SNODE]             # storage -> logical row


def _build():
    nc = bacc.Bacc("TRN2", target_bir_lowering=False, debug=False)

    # xt[g*2+h]: [cc, (storage stream, ck, col)] transformed input halves
    xt = nc.dram_tensor("xt", [NG * 2, 128, HCOLS], mybir.dt.bfloat16,
                        kind="ExternalInput")
    # cwr[dck]: [cc, (s*8+ck)*128+dd] conv lhsT blocks, storage order
    cwr = nc.dram_tensor("cwr", [NCH, 128, 6 * NCH * 128], mybir.dt.bfloat16,
                         kind="ExternalInput")
    # gwr[eck]: [dd, dck*128+ee] gate lhsT blocks
    gwr = nc.dram_tensor("gwr", [NCH, 128, NCH * 128], mybir.dt.bfloat16,
                         kind="ExternalInput")
    cb = nc.dram_tensor("cb", [128, NCH], mybir.dt.float32, kind="ExternalInput")
    gb = nc.dram_tensor("gb", [128, NCH], mybir.dt.float32, kind="ExternalInput")
    outT = nc.dram_tensor("outT", [D, RC], mybir.dt.float32, kind="ExternalOutput")

    with tile.TileContext(nc) as tc:
        with (
            tc.tile_pool(name="consts", bufs=1) as consts,
            tc.tile_pool(name="xtp", bufs=1) as xtp,
            tc.tile_pool(name="cnn", bufs=1) as cnnp,
            tc.tile_pool(name="ms", bufs=2) as msp,
            tc.tile_pool(name="work", bufs=2) as work,
            tc.tile_pool(name="gout", bufs=4) as gout,
            tc.tile_pool(name="psum", bufs=2, space="PSUM") as psum,
        ):
            cwr_sb = [None] * NCH
            xt_sb = [None, None]

            def load_xt_half(g, h):
                # scalar (ACT) HWDGE queue: parallel to the sync queue
                t = xtp.tile([128, HCOLS], mybir.dt.bfloat16, tag=f"xt{h}",
                             name=f"xt{h}_g{g}")
                nc.scalar.dma_start(t[:], xt[g * 2 + h])
                xt_sb[h] = t

            # ---- DMA ramp (first-use order, per-stream pieces for dck0) ----
            cw0 = consts.tile([128, 6 * NCH * 128], mybir.dt.bfloat16, tag="cw0")
            cwr_sb[0] = cw0
            xt_sb[0] = xtp.tile([128, HCOLS], mybir.dt.bfloat16, tag="xt0",
                                name="xt0_g0")
            xt_sb[1] = xtp.tile([128, HCOLS], mybir.dt.bfloat16, tag="xt1",
                                name="xt1_g0")
            # dual-queue ramp: cw0 stream pieces + cwr1-7 + gwr on the sync
            # (SP) queue; xt pieces + cb/gb on the scalar (ACT) queue.  Each
            # DMA instruction costs ~650ns of queue issue time, so splitting
            # halves the serial issue latency of the ramp.
            SB = NCH * 128                       # cwr cols per stream (1024)
            for s in range(6):
                h = 0 if s < 3 else 1
                nc.sync.dma_start(cw0[:, s * SB:(s + 1) * SB],
                                  cwr[0][:, s * SB:(s + 1) * SB])
                o = SOFF_X[s]
                nc.sync.dma_start(xt_sb[h][:, o:o + NCH * WID[s]],
                                  xt[h][:, o:o + NCH * WID[s]])
            # tiny bias loads on the scalar queue (off the critical sync FIFO)
            cb_sb = consts.tile([128, NCH], mybir.dt.float32, tag="cb")
            nc.scalar.dma_start(cb_sb[:], cb[:])
            gb_sb = consts.tile([128, NCH], mybir.dt.float32, tag="gb")
            nc.scalar.dma_start(gb_sb[:], gb[:])
            for dck in range(1, NCH):
                cwt = consts.tile([128, 6 * NCH * 128], mybir.dt.bfloat16,
                                  tag=f"cw{dck}")
                cwr_sb[dck] = cwt
                nc.sync.dma_start(cwt[:], cwr[dck])
            gwr_sb = []
            for eck in range(NCH):
                t = consts.tile([128, NCH * 128], mybir.dt.bfloat16, tag=f"gw{eck}")
                nc.sync.dma_start(t[:], gwr[eck])
                gwr_sb.append(t)

            # ---- HAM warmup over the preamble->first-data window ----
            scr = consts.tile([128, 256], mybir.dt.bfloat16, tag="scr")
            nc.gpsimd.memset(scr[:], 0.0)
            # enough cold matmuls (~5.5us) to bridge clock-start -> first
            # conv data; keeps the HAM busy-window unbroken so real chains
            # run at 8/8 from the start
            wps = psum.tile([128, 2048], mybir.dt.float32, tag="cps")
            for _ in range(14):
                nc.tensor.matmul(wps[:, :256], scr[:, :128], scr[:, :256],
                                 start=True, stop=True)

            cnn_t = [[None] * NCH for _ in range(NG)]

            def conv_unit(g, dck, pad_intra=False, pad_end=0):
                ps = psum.tile([128, 2048], mybir.dt.float32, tag="cps")
                for s in range(6):
                    h = 0 if s < 3 else 1
                    w = WID[s]
                    for ck in range(NCH):
                        nc.tensor.matmul(
                            ps[:, POFF[s]:POFF[s] + w],
                            cwr_sb[dck][:, (s * NCH + ck) * 128:
                                        (s * NCH + ck + 1) * 128],
                            xt_sb[h][:, SOFF_X[s] + ck * w:
                                     SOFF_X[s] + (ck + 1) * w],
                            start=(ck == 0),
                            stop=(ck == NCH - 1),
                        )
                    # anti-idle fillers into this slot's unused pad cols
                    # (bank3 tail): keep HAM at 8/8 while the ramp DMA
                    # trickles chain inputs in; never read.
                    npad = (3 if pad_intra else 0) if s < 5 else pad_end
                    for _ in range(npad):
                        nc.tensor.matmul(ps[:, 1824:2048], scr[:, :128],
                                         scr[:, :224], start=True, stop=True)
                combine(g, dck, ps)

            def combine(g, dck, ps):
                cbs = cb_sb[:, dck:dck + 1]
                bf = mybir.dt.bfloat16
                mS = msp.tile([128, 1536], bf, tag="ms")

                # copy-first: one ACT copy per psum bank, in chain order;
                # the slot frees after c4 (~1us after the unit's last MM)
                nc.scalar.activation(mS[:, 0:480], ps[:, 0:480], AF.Copy)
                nc.scalar.activation(mS[:, 480:960], ps[:, 512:992], AF.Copy)
                nc.scalar.activation(mS[:, 960:1248], ps[:, 1024:1312], AF.Copy)
                nc.scalar.activation(mS[:, 1248:1536], ps[:, 1536:1824], AF.Copy)
                m1 = mS[:, MOFF[0]:MOFF[0] + 288]
                m3 = mS[:, MOFF[1]:MOFF[1] + 192]
                m2 = mS[:, MOFF[2]:MOFF[2] + 288]
                m4 = mS[:, MOFF[3]:MOFF[3] + 192]
                m0 = mS[:, MOFF[4]:MOFF[4] + 288]
                m5 = mS[:, MOFF[5]:MOFF[5] + 288]

                def st(tag, n):
                    return work.tile([128, n], bf, tag=tag, name=tag)

                cnn = cnnp.tile([128, GN], bf, tag=f"cnn{g}_{dck}")
                cnn_t[g][dck] = cnn
                v = cnn[:].rearrange("p (t w) -> p t w", w=GWN)

                def outab(t0):
                    # A-tile output t0 and B-tile output t0+4: [128, 2, 96]
                    return v[:, t0:t0 + 5:4]

                # u_k = b^k m3 + cb on ACT; h_k = (-1/2)^k m4 on DVE
                u1 = st("u1", AB)
                nc.scalar.activation(u1[:], m3, AF.Identity, bias=cbs, scale=2.0)
                u2 = st("u2", AB)
                nc.scalar.activation(u2[:], m3, AF.Identity, bias=cbs, scale=4.0)
                u3 = st("u3", AB)
                nc.scalar.activation(u3[:], m3, AF.Identity, bias=cbs, scale=8.0)
                s1 = st("s1", 288); nc.vector.tensor_add(s1[:], m1, m2)
                d1 = st("d1", 288); nc.vector.tensor_sub(d1[:], m1, m2)
                h1 = st("h1", AB); nc.vector.tensor_scalar_mul(h1[:], m4, -0.5)
                h2 = st("h2", AB); nc.vector.tensor_scalar_mul(h2[:], m4, 0.25)
                h3 = st("h3", AB); nc.vector.tensor_scalar_mul(h3[:], m4, -0.125)
                a0 = st("a0", AB); nc.vector.tensor_add(a0[:], m3, m4)
                t0 = st("t0", AB); nc.vector.tensor_scalar_add(t0[:], a0[:], cbs)
                P = st("P", 288)
                nc.gpsimd.tensor_add(P[:], m0, s1[:])
                nc.gpsimd.tensor_add(outab(0), P[:, :AB], t0[:])          # y0
                w1 = st("w1", AB); nc.gpsimd.tensor_add(w1[:], d1[:, :AB], u1[:])
                nc.gpsimd.tensor_add(outab(1), w1[:], h1[:])              # y1
                w2 = st("w2", AB); nc.gpsimd.tensor_add(w2[:], s1[:, :AB], u2[:])
                nc.gpsimd.tensor_add(outab(2), w2[:], h2[:])              # y2
                w3 = st("w3", AB); nc.vector.tensor_add(w3[:], d1[:, :AB], u3[:])
                x3 = st("x3", AB); nc.vector.tensor_add(x3[:], w3[:], h3[:])
                nc.gpsimd.tensor_add(outab(3), x3[:], m5[:, :AB])         # y3
                # tails: y8 = P[T]+cb ; y9 = d1[T] + m5[T] + cb
                nc.vector.tensor_scalar_add(v[:, 8, :], P[:, TL:TH], cbs)
                e9 = st("e9", GWN)
                nc.vector.tensor_add(e9[:], d1[:, TL:TH], m5[:, TL:TH])
                nc.vector.tensor_scalar_add(v[:, 9, :], e9[:], cbs)

            def gate_quad(g, eck0, tail=False):
                # 4 chains (eck0,c0),(eck0,c1),(eck0+1,c0),(eck0+1,c1) into
                # banks 0-3 of one psum slot, accumulated dck-MAJOR: the
                # cnn[dck=7]-dependent matmuls land ~5.7us of PE work after
                # a conv phase end (covers the trailing combine backlog),
                # and the deep chain pipeline avoids psum-WAR stalls.
                # tail=True (very last quad): chain 3 runs alone after the
                # other three so only its short split epilog trails the
                # kernel's last matmul.
                chains = [(eck0, 0), (eck0, 1), (eck0 + 1, 0), (eck0 + 1, 1)]

                def epilog(pst, q, eck, c, split=False):
                    gt = gout.tile([128, 480], mybir.dt.bfloat16, tag="gate")
                    ot = gout.tile([128, 480], mybir.dt.float32, tag="out")
                    chunks = ((0, 240), (240, 480)) if split else ((0, 480),)
                    for lo, hi in chunks:
                        nc.scalar.activation(gt[:, lo:hi],
                                             pst[:, q * 512 + lo:q * 512 + hi],
                                             AF.Sigmoid,
                                             bias=gb_sb[:, eck:eck + 1])
                        nc.vector.tensor_mul(ot[:, lo:hi],
                                             cnn_t[g][eck][:, c * 480 + lo:
                                                           c * 480 + hi],
                                             gt[:, lo:hi])
                        nc.sync.dma_start(
                            outT[eck * 128:(eck + 1) * 128,
                                 g * GN + c * 480 + lo:g * GN + c * 480 + hi],
                            ot[:, lo:hi])

                if tail:
                    # chains 0-2 dck-major in ps2; chain 3 in its OWN
                    # rotated tile with matmuls emitted BEFORE the other
                    # epilogs (tile-granular WAR tracking would stall them
                    # on the sigmoids reading ps2), so only chain 3's short
                    # split epilog trails the kernel's final matmul.
                    ps2 = psum.tile([128, 2048], mybir.dt.float32, tag="cps")
                    for dck in range(NCH):
                        for q in range(3):
                            eck, c = chains[q]
                            nc.tensor.matmul(
                                ps2[:, q * 512:q * 512 + 480],
                                gwr_sb[eck][:, dck * 128:(dck + 1) * 128],
                                cnn_t[g][dck][:, c * 480:(c + 1) * 480],
                                start=(dck == 0),
                                stop=(dck == NCH - 1),
                            )
                    ps3 = psum.tile([128, 2048], mybir.dt.float32, tag="cps",
                                    name="tailq3")
                    eck3, c3 = chains[3]
                    for dck in range(NCH):
                        nc.tensor.matmul(
                            ps3[:, :480],
                            gwr_sb[eck3][:, dck * 128:(dck + 1) * 128],
                            cnn_t[g][dck][:, c3 * 480:(c3 + 1) * 480],
                            start=(dck == 0),
                            stop=(dck == NCH - 1),
                        )
                    for q in range(3):
                        epilog(ps2, q, *chains[q])
                    epilog(ps3, 0, eck3, c3, split=True)
                else:
                    ps2 = psum.tile([128, 2048], mybir.dt.float32, tag="cps")
                    for dck in range(NCH):
                        for q, (eck, c) in enumerate(chains):
                            nc.tensor.matmul(
                                ps2[:, q * 512:q * 512 + 480],
                                gwr_sb[eck][:, dck * 128:(dck + 1) * 128],
                                cnn_t[g][dck][:, c * 480:(c + 1) * 480],
                                start=(dck == 0),
                                stop=(dck == NCH - 1),
                            )
                    for q, (eck, c) in enumerate(chains):
                        epilog(ps2, q, eck, c)

            # phase g0 convs (cwr streams in behind; early units pad their
            # DMA-trickle idle with anti-idle matmuls)
            for dck in range(NCH):
                conv_unit(0, dck, pad_intra=(dck < 2),
                          pad_end=(4 if dck < 4 else 0))
            # xt g1 reloads during the g0 gate phase (WAR on the xt slots
            # releases at g0-conv end; the 25us gate window covers it)
            load_xt_half(1, 0)
            load_xt_half(1, 1)
            for eck0 in range(0, NCH, 2):
                gate_quad(0, eck0)
            for dck in range(NCH):
                conv_unit(1, dck)
            for eck0 in range(0, NCH, 2):
                gate_quad(1, eck0, tail=(eck0 == NCH - 2))
    nc.compile()
    return nc


def _prep_weights(conv_w, conv_b, gate_w, gate_b):
    W0, W1, W2 = [conv_w[:, :, k].astype(np.float64) for k in range(3)]
    g = []
    for b in SNODE:
        g.append(W2 if b is INF else W0 + b * W1 + b * b * W2)
    garr = np.stack(g)                                   # [6 storage, Do, Di]
    gv = garr.reshape(6, NCH, 128, NCH, 128)             # [s, dck, dd, ck, cc]
    cw_host = np.ascontiguousarray(gv.transpose(1, 4, 0, 3, 2)).reshape(
        NCH, 128, 6 * NCH * 128).astype(BF16)
    gwt = gate_w.T.reshape(NCH, 128, NCH, 128)           # [dck, dd, eck, ee]
    gw_host = np.ascontiguousarray(gwt.transpose(2, 1, 0, 3)).reshape(
        NCH, 128, NCH * 128).astype(BF16)
    cb_host = np.ascontiguousarray(conv_b.reshape(NCH, 128).T).astype(np.float32)
    gb_host = np.ascontiguousarray(gate_b.reshape(NCH, 128).T).astype(np.float32)
    return cw_host, gw_host, cb_host, gb_host


def _prep_core_x(x_shard):
    # x_shard [BC, T, D] -> xt [NG*2, 128, HCOLS]
    xw = x_shard.reshape(NWIN, W, D).astype(np.float64)
    xp = np.pad(xw, ((0, 0), (1, 1), (0, 0)))            # [192, 12, D]
    xt_host = np.empty((NG * 2, 128, HCOLS), BF16)
    for g in range(NG):
        ws = xp[g * GWN:(g + 1) * GWN]                   # [96, 12, D]
        xA = np.einsum('ji,wic->jwc', BA, ws[:, 0:6])    # [6, 96, D]
        xB = np.einsum('ji,wic->jwc', BA, ws[:, 4:10])
        xT = np.einsum('ji,wic->jwc', BT4, ws[:, 8:12])  # [4, 96, D]
        for s in range(6):
            j = LOGI[s]
            parts = [xA[j], xB[j]]
            if s in TROW:
                parts.append(xT[TROW[s]])
            S = np.concatenate(parts, axis=0)            # [WID[s], D]
            blk = S.T.reshape(NCH, 128, WID[s]).transpose(1, 0, 2)
            h = 0 if s < 3 else 1
            o = SOFF_X[s]
            xt_host[g * 2 + h, :, o:o + NCH * WID[s]] = \
                blk.reshape(128, NCH * WID[s]).astype(BF16)
    return xt_host


def _unshard_core(o):
    # o: [D, RC] cols ordered (g, t, w); group g == local batch row,
    # window w at in-window position t -> time w*10+t
    return np.ascontiguousarray(
        o.reshape(D, NG, W, GWN).transpose(1, 3, 2, 0).reshape(BC, T, D))


_NC_CACHE = None


def _prep_in_maps(x, conv_w, conv_b, gate_w, gate_b):
    cw_host, gw_host, cb_host, gb_host = _prep_weights(
        conv_w, conv_b, gate_w, gate_b)
    return [
        {"xt": _prep_core_x(x[BC * i:BC * (i + 1)]), "cwr": cw_host,
         "gwr": gw_host, "cb": cb_host, "gb": gb_host}
        for i in range(NCORES)
    ]


def kernel(x, conv_w, conv_b, gate_w, gate_b):
    global _NC_CACHE
    x = np.asarray(x, np.float32)
    conv_w = np.asarray(conv_w, np.float32)
    conv_b = np.asarray(conv_b, np.float32)
    gate_w = np.asarray(gate_w, np.float32)
    gate_b = np.asarray(gate_b, np.float32)

    in_maps = _prep_in_maps(x, conv_w, conv_b, gate_w, gate_b)
    if _NC_CACHE is None:
        _NC_CACHE = _build()
    res = run_bass_kernel_spmd(_NC_CACHE, in_maps,
                               core_ids=list(range(NCORES))).results

    out = np.empty((B, T, D), np.float32)
    for i in range(NCORES):
        out[BC * i:BC * (i + 1)] = _unshard_core(np.asarray(res[i]["outT"]))
    return out


# revision 27
# speedup vs baseline: 1.0179x; 1.0109x over previous
"""Trainium2 Bass kernel for windowed Conv1d(k=3) + sigmoid gating.

Reference (B=16, T=960, D=1024, W=10): windows of 10 conv'd independently
with per-window zero pad 1:
    cnn[t, d] = sum_{k,c} conv_w[d, c, k] * xpad[t + k, c]
    out = cnn * sigmoid(cnn @ gate_w.T + gate_b)

Strategy: data parallel over 8 cores (2 batches / 192 windows / core).
Hybrid Winograd conv: two F(4,3) tiles (outputs 0-3 from xp[0:6], outputs
4-7 from xp[4:10]) at points {0,1,-1,2,-1/2,inf} plus an F(2,3) tail
(outputs 8,9 from xp[8:12]) at points {0,1,-1,inf} folded into the same
weight streams => 16 muls/window (vs 30 direct).  Streams use canonical
Vandermonde weights g_b = [1,b,b^2].W (normalization in the host-side
V^{-T} input transforms).

Per core: 2 groups of 96 windows (group == local batch row).  Per
(group, dck): 6 accumulation chains into one 4-bank PSUM slot packed
{m1+m3 | m2+m4 | m0 | m5} (chains may not cross banks).  The combine is
copy-first: ACT copies each bank to bf16 SBUF as its chains finish
(frees the slot fast => no PE stall at unit boundaries), then the A^T
combine runs in bf16 on DVE/GpSimd/ACT with ~1.6us/unit slack per engine.
Tails use a single class (no rotation): stream columns are [A|B|T].

Phases: g0 convs -> all g0 gates -> g1 convs -> g1 gates; xt[g1]
reloads into the xt slots during the g0 gate phase (25us window);
gwr is ordered after cwr in the DMA stream (first-use order).  A short
cold warmup (~12 matmuls) covers the preamble->first-data window.
"""

import numpy as np
import ml_dtypes

import concourse.bacc as bacc
import concourse.bass as bass
import concourse.tile as tile
from concourse import mybir
from concourse.bass_utils import run_bass_kernel_spmd

BF16 = ml_dtypes.bfloat16

B, T, D, W = 16, 960, 1024, 10
NCORES = 8
BC = B // NCORES             # batches per core (2)
NWIN = BC * T // W           # windows per core (192)
RC = NWIN * W                # output rows per core (1920)
NG = 2                       # groups per core (group == local batch row)
GWN = NWIN // NG             # windows per group (96)
GN = GWN * W                 # output cols per group (960)
NCH = D // 128               # channel chunks (8)
AF = mybir.ActivationFunctionType

INF = "inf"
PTS = [0.0, 1.0, -1.0, 2.0, -0.5, INF]          # logical nodes
TPTS = [0.0, 1.0, -1.0, INF]                     # tail nodes
# storage (chain) order of streams: nodes per storage slot
SNODE = [1.0, 2.0, -1.0, -0.5, 0.0, INF]         # m1 m3 m2 m4 m0 m5
SW = [3, 2, 3, 2, 3, 3]                          # width in 96-col blocks
WID = [w * GWN for w in SW]                      # [288,192,288,192,288,288]
# tail row in TPTS for storage streams that carry a tail block
TROW = {0: 1, 2: 2, 4: 0, 5: 3}
# per-half storage streams (matches xt half layout and cwr layout)
HSTREAMS = [(0, 1, 2), (3, 4, 5)]
HCOLS = 6144                                     # cols per xt half (768*8)
# psum col offset per storage stream (4 banks of 512 fp32)
POFF = [0, 288, 512, 800, 1024, 1536]
# bf16 mS col offset per storage stream (packed, 1536 cols)
MOFF = [0, 288, 480, 768, 960, 1248]
# cwr col offset of (storage stream, ck) block
SOFF_X = []                                      # xt col offset per stream
_off = 0
for _s in range(6):
    SOFF_X.append(_off if _s < 3 else _off - HCOLS)
    _off += WID[_s] * NCH
AB = 2 * GWN                                     # 192 (A|B block)
TL, TH = AB, AB + GWN                            # tail block 192:288


def _vinv_T(points):
    n = len(points)
    V = np.zeros((n, n))
    for j, b in enumerate(points):
        if b is INF:
            V[j, n - 1] = 1.0
        else:
            V[j] = [float(b) ** i for i in range(n)]
    return np.linalg.inv(V).T


BA = _vinv_T(PTS)
BT4 = _vinv_T(TPTS)
LOGI = [PTS.index(b) for b in SNODE]             # storage -> logical row


def _build():
    nc = bacc.Bacc("TRN2", target_bir_lowering=False, debug=False)

    # xt[g*2+h]: [cc, (storage stream, ck, col)] transformed input halves
    xt = nc.dram_tensor("xt", [NG * 2, 128, HCOLS], mybir.dt.bfloat16,
                        kind="ExternalInput")
    # cwr[dck]: [cc, (s*8+ck)*128+dd] conv lhsT blocks, storage order
    cwr = nc.dram_tensor("cwr", [NCH, 128, 6 * NCH * 128], mybir.dt.bfloat16,
                         kind="ExternalInput")
    # gwr[eck]: [dd, dck*128+ee] gate lhsT blocks
    gwr = nc.dram_tensor("gwr", [NCH, 128, NCH * 128], mybir.dt.bfloat16,
                         kind="ExternalInput")
    cb = nc.dram_tensor("cb", [128, NCH], mybir.dt.float32, kind="ExternalInput")
    gb = nc.dram_tensor("gb", [128, NCH], mybir.dt.float32, kind="ExternalInput")
    outT = nc.dram_tensor("outT", [D, RC], mybir.dt.float32, kind="ExternalOutput")

    with tile.TileContext(nc) as tc:
        with (
            tc.tile_pool(name="consts", bufs=1) as consts,
            tc.tile_pool(name="xtp", bufs=1) as xtp,
            tc.tile_pool(name="cnn", bufs=1) as cnnp,
            tc.tile_pool(name="ms", bufs=2) as msp,
            tc.tile_pool(name="work", bufs=2) as work,
            tc.tile_pool(name="gout", bufs=4) as gout,
            tc.tile_pool(name="psum", bufs=2, space="PSUM") as psum,
        ):
            cwr_sb = [None] * NCH
            xt_sb = [None, None]

            def load_xt_half(g, h):
                # scalar (ACT) HWDGE queue: parallel to the sync queue
                t = xtp.tile([128, HCOLS], mybir.dt.bfloat16, tag=f"xt{h}",
                             name=f"xt{h}_g{g}")
                nc.scalar.dma_start(t[:], xt[g * 2 + h])
                xt_sb[h] = t

            # ---- DMA ramp (first-use order, per-stream pieces for dck0) ----
            cw0 = consts.tile([128, 6 * NCH * 128], mybir.dt.bfloat16, tag="cw0")
            cwr_sb[0] = cw0
            xt_sb[0] = xtp.tile([128, HCOLS], mybir.dt.bfloat16, tag="xt0",
                                name="xt0_g0")
            xt_sb[1] = xtp.tile([128, HCOLS], mybir.dt.bfloat16, tag="xt1",
                                name="xt1_g0")
            # dual-queue ramp: cw0 stream pieces + cwr1-7 + gwr on the sync
            # (SP) queue; xt pieces + cb/gb on the scalar (ACT) queue.  Each
            # DMA instruction costs ~650ns of queue issue time, so splitting
            # halves the serial issue latency of the ramp.
            SB = NCH * 128                       # cwr cols per stream (1024)
            for s in range(6):
                h = 0 if s < 3 else 1
                nc.sync.dma_start(cw0[:, s * SB:(s + 1) * SB],
                                  cwr[0][:, s * SB:(s + 1) * SB])
                o = SOFF_X[s]
                nc.sync.dma_start(xt_sb[h][:, o:o + NCH * WID[s]],
                                  xt[h][:, o:o + NCH * WID[s]])
            # tiny bias loads on the scalar queue (off the critical sync FIFO)
            cb_sb = consts.tile([128, NCH], mybir.dt.float32, tag="cb")
            nc.scalar.dma_start(cb_sb[:], cb[:])
            gb_sb = consts.tile([128, NCH], mybir.dt.float32, tag="gb")
            nc.scalar.dma_start(gb_sb[:], gb[:])
            for dck in range(1, NCH):
                cwt = consts.tile([128, 6 * NCH * 128], mybir.dt.bfloat16,
                                  tag=f"cw{dck}")
                cwr_sb[dck] = cwt
                nc.sync.dma_start(cwt[:], cwr[dck])
            gwr_sb = []
            for eck in range(NCH):
                t = consts.tile([128, NCH * 128], mybir.dt.bfloat16, tag=f"gw{eck}")
                nc.sync.dma_start(t[:], gwr[eck])
                gwr_sb.append(t)

            # ---- HAM warmup over the preamble->first-data window ----
            scr = consts.tile([128, 256], mybir.dt.bfloat16, tag="scr")
            nc.gpsimd.memset(scr[:], 0.0)
            # enough cold matmuls (~5.5us) to bridge clock-start -> first
            # conv data; keeps the HAM busy-window unbroken so real chains
            # run at 8/8 from the start
            wps = psum.tile([128, 2048], mybir.dt.float32, tag="cps")
            for _ in range(22):
                nc.tensor.matmul(wps[:, :256], scr[:, :128], scr[:, :256],
                                 start=True, stop=True)

            cnn_t = [[None] * NCH for _ in range(NG)]

            def conv_unit(g, dck, pad_intra=False, pad_end=0):
                ps = psum.tile([128, 2048], mybir.dt.float32, tag="cps")
                for s in range(6):
                    h = 0 if s < 3 else 1
                    w = WID[s]
                    for ck in range(NCH):
                        nc.tensor.matmul(
                            ps[:, POFF[s]:POFF[s] + w],
                            cwr_sb[dck][:, (s * NCH + ck) * 128:
                                        (s * NCH + ck + 1) * 128],
                            xt_sb[h][:, SOFF_X[s] + ck * w:
                                     SOFF_X[s] + (ck + 1) * w],
                            start=(ck == 0),
                            stop=(ck == NCH - 1),
                        )
                    # anti-idle fillers into this slot's unused pad cols
                    # (bank3 tail): keep HAM at 8/8 while the ramp DMA
                    # trickles chain inputs in; never read.
                    npad = (3 if pad_intra else 0) if s < 5 else pad_end
                    for _ in range(npad):
                        nc.tensor.matmul(ps[:, 1824:2048], scr[:, :128],
                                         scr[:, :224], start=True, stop=True)
                combine(g, dck, ps)

            def combine(g, dck, ps):
                cbs = cb_sb[:, dck:dck + 1]
                bf = mybir.dt.bfloat16
                mS = msp.tile([128, 1536], bf, tag="ms")

                # copy-first: one ACT copy per psum bank, in chain order;
                # the slot frees after c4 (~1us after the unit's last MM)
                nc.scalar.activation(mS[:, 0:480], ps[:, 0:480], AF.Copy)
                nc.scalar.activation(mS[:, 480:960], ps[:, 512:992], AF.Copy)
                nc.scalar.activation(mS[:, 960:1248], ps[:, 1024:1312], AF.Copy)
                nc.scalar.activation(mS[:, 1248:1536], ps[:, 1536:1824], AF.Copy)
                m1 = mS[:, MOFF[0]:MOFF[0] + 288]
                m3 = mS[:, MOFF[1]:MOFF[1] + 192]
                m2 = mS[:, MOFF[2]:MOFF[2] + 288]
                m4 = mS[:, MOFF[3]:MOFF[3] + 192]
                m0 = mS[:, MOFF[4]:MOFF[4] + 288]
                m5 = mS[:, MOFF[5]:MOFF[5] + 288]

                def st(tag, n):
                    return work.tile([128, n], bf, tag=tag, name=tag)

                cnn = cnnp.tile([128, GN], bf, tag=f"cnn{g}_{dck}")
                cnn_t[g][dck] = cnn
                v = cnn[:].rearrange("p (t w) -> p t w", w=GWN)

                def outab(t0):
                    # A-tile output t0 and B-tile output t0+4: [128, 2, 96]
                    return v[:, t0:t0 + 5:4]

                # u_k = b^k m3 + cb on ACT; h_k = (-1/2)^k m4 on DVE
                u1 = st("u1", AB)
                nc.scalar.activation(u1[:], m3, AF.Identity, bias=cbs, scale=2.0)
                u2 = st("u2", AB)
                nc.scalar.activation(u2[:], m3, AF.Identity, bias=cbs, scale=4.0)
                u3 = st("u3", AB)
                nc.scalar.activation(u3[:], m3, AF.Identity, bias=cbs, scale=8.0)
                s1 = st("s1", 288); nc.vector.tensor_add(s1[:], m1, m2)
                d1 = st("d1", 288); nc.vector.tensor_sub(d1[:], m1, m2)
                h1 = st("h1", AB); nc.vector.tensor_scalar_mul(h1[:], m4, -0.5)
                h2 = st("h2", AB); nc.vector.tensor_scalar_mul(h2[:], m4, 0.25)
                h3 = st("h3", AB); nc.vector.tensor_scalar_mul(h3[:], m4, -0.125)
                a0 = st("a0", AB); nc.vector.tensor_add(a0[:], m3, m4)
                t0 = st("t0", AB); nc.vector.tensor_scalar_add(t0[:], a0[:], cbs)
                P = st("P", 288)
                nc.gpsimd.tensor_add(P[:], m0, s1[:])
                nc.gpsimd.tensor_add(outab(0), P[:, :AB], t0[:])          # y0
                w1 = st("w1", AB); nc.gpsimd.tensor_add(w1[:], d1[:, :AB], u1[:])
                nc.gpsimd.tensor_add(outab(1), w1[:], h1[:])              # y1
                w2 = st("w2", AB); nc.gpsimd.tensor_add(w2[:], s1[:, :AB], u2[:])
                nc.gpsimd.tensor_add(outab(2), w2[:], h2[:])              # y2
                w3 = st("w3", AB); nc.vector.tensor_add(w3[:], d1[:, :AB], u3[:])
                x3 = st("x3", AB); nc.vector.tensor_add(x3[:], w3[:], h3[:])
                nc.gpsimd.tensor_add(outab(3), x3[:], m5[:, :AB])         # y3
                # tails: y8 = P[T]+cb ; y9 = d1[T] + m5[T] + cb
                nc.vector.tensor_scalar_add(v[:, 8, :], P[:, TL:TH], cbs)
                e9 = st("e9", GWN)
                nc.vector.tensor_add(e9[:], d1[:, TL:TH], m5[:, TL:TH])
                nc.vector.tensor_scalar_add(v[:, 9, :], e9[:], cbs)

            def gate_quad(g, eck0, tail=False):
                # 4 chains (eck0,c0),(eck0,c1),(eck0+1,c0),(eck0+1,c1) into
                # banks 0-3 of one psum slot, accumulated dck-MAJOR: the
                # cnn[dck=7]-dependent matmuls land ~5.7us of PE work after
                # a conv phase end (covers the trailing combine backlog),
                # and the deep chain pipeline avoids psum-WAR stalls.
                # tail=True (very last quad): chain 3 runs alone after the
                # other three so only its short split epilog trails the
                # kernel's last matmul.
                chains = [(eck0, 0), (eck0, 1), (eck0 + 1, 0), (eck0 + 1, 1)]

                def epilog(pst, q, eck, c, split=False):
                    gt = gout.tile([128, 480], mybir.dt.bfloat16, tag="gate")
                    ot = gout.tile([128, 480], mybir.dt.float32, tag="out")
                    chunks = ((0, 240), (240, 480)) if split else ((0, 480),)
                    for lo, hi in chunks:
                        nc.scalar.activation(gt[:, lo:hi],
                                             pst[:, q * 512 + lo:q * 512 + hi],
                                             AF.Sigmoid,
                                             bias=gb_sb[:, eck:eck + 1])
                        nc.vector.tensor_mul(ot[:, lo:hi],
                                             cnn_t[g][eck][:, c * 480 + lo:
                                                           c * 480 + hi],
                                             gt[:, lo:hi])
                        nc.sync.dma_start(
                            outT[eck * 128:(eck + 1) * 128,
                                 g * GN + c * 480 + lo:g * GN + c * 480 + hi],
                            ot[:, lo:hi])

                if tail:
                    # chains 0-2 dck-major in ps2; chain 3 in its OWN
                    # rotated tile with matmuls emitted BEFORE the other
                    # epilogs (tile-granular WAR tracking would stall them
                    # on the sigmoids reading ps2), so only chain 3's short
                    # split epilog trails the kernel's final matmul.
                    ps2 = psum.tile([128, 2048], mybir.dt.float32, tag="cps")
                    for dck in range(NCH):
                        for q in range(3):
                            eck, c = chains[q]
                            nc.tensor.matmul(
                                ps2[:, q * 512:q * 512 + 480],
                                gwr_sb[eck][:, dck * 128:(dck + 1) * 128],
                                cnn_t[g][dck][:, c * 480:(c + 1) * 480],
                                start=(dck == 0),
                                stop=(dck == NCH - 1),
                            )
                    ps3 = psum.tile([128, 2048], mybir.dt.float32, tag="cps",
                                    name="tailq3")
                    eck3, c3 = chains[3]
                    for dck in range(NCH):
                        nc.tensor.matmul(
                            ps3[:, :480],
                            gwr_sb[eck3][:, dck * 128:(dck + 1) * 128],
                            cnn_t[g][dck][:, c3 * 480:(c3 + 1) * 480],
                            start=(dck == 0),
                            stop=(dck == NCH - 1),
                        )
                    for q in range(3):
                        epilog(ps2, q, *chains[q])
                    epilog(ps3, 0, eck3, c3, split=True)
                else:
                    ps2 = psum.tile([128, 2048], mybir.dt.float32, tag="cps")
                    for dck in range(NCH):
                        for q, (eck, c) in enumerate(chains):
                            nc.tensor.matmul(
                                ps2[:, q * 512:q * 512 + 480],
                                gwr_sb[eck][:, dck * 128:(dck + 1) * 128],
                                cnn_t[g][dck][:, c * 480:(c + 1) * 480],
                                start=(dck == 0),
                                stop=(dck == NCH - 1),
                            )
                    for q, (eck, c) in enumerate(chains):
                        epilog(ps2, q, eck, c)

            # phase g0 convs (cwr streams in behind; early units pad their
            # DMA-trickle idle with anti-idle matmuls)
            for dck in range(NCH):
                conv_unit(0, dck, pad_intra=(dck < 2),
                          pad_end=(4 if dck < 4 else 0))
            # xt g1 reloads during the g0 gate phase (WAR on the xt slots
            # releases at g0-conv end; the 25us gate window covers it)
            load_xt_half(1, 0)
            load_xt_half(1, 1)
            for eck0 in range(0, NCH, 2):
                gate_quad(0, eck0)
            for dck in range(NCH):
                conv_unit(1, dck)
            for eck0 in range(0, NCH, 2):
                gate_quad(1, eck0, tail=(eck0 == NCH - 2))
    nc.compile()
    return nc


def _prep_weights(conv_w, conv_b, gate_w, gate_b):
    W0, W1, W2 = [conv_w[:, :, k].astype(np.float64) for k in range(3)]
    g = []
    for b in SNODE:
        g.append(W2 if b is INF else W0 + b * W1 + b * b * W2)
    garr = np.stack(g)                                   # [6 storage, Do, Di]
    gv = garr.reshape(6, NCH, 128, NCH, 128)             # [s, dck, dd, ck, cc]
    cw_host = np.ascontiguousarray(gv.transpose(1, 4, 0, 3, 2)).reshape(
        NCH, 128, 6 * NCH * 128).astype(BF16)
    gwt = gate_w.T.reshape(NCH, 128, NCH, 128)           # [dck, dd, eck, ee]
    gw_host = np.ascontiguousarray(gwt.transpose(2, 1, 0, 3)).reshape(
        NCH, 128, NCH * 128).astype(BF16)
    cb_host = np.ascontiguousarray(conv_b.reshape(NCH, 128).T).astype(np.float32)
    gb_host = np.ascontiguousarray(gate_b.reshape(NCH, 128).T).astype(np.float32)
    return cw_host, gw_host, cb_host, gb_host


def _prep_core_x(x_shard):
    # x_shard [BC, T, D] -> xt [NG*2, 128, HCOLS]
    xw = x_shard.reshape(NWIN, W, D).astype(np.float64)
    xp = np.pad(xw, ((0, 0), (1, 1), (0, 0)))            # [192, 12, D]
    xt_host = np.empty((NG * 2, 128, HCOLS), BF16)
    for g in range(NG):
        ws = xp[g * GWN:(g + 1) * GWN]                   # [96, 12, D]
        xA = np.einsum('ji,wic->jwc', BA, ws[:, 0:6])    # [6, 96, D]
        xB = np.einsum('ji,wic->jwc', BA, ws[:, 4:10])
        xT = np.einsum('ji,wic->jwc', BT4, ws[:, 8:12])  # [4, 96, D]
        for s in range(6):
            j = LOGI[s]
            parts = [xA[j], xB[j]]
            if s in TROW:
                parts.append(xT[TROW[s]])
            S = np.concatenate(parts, axis=0)            # [WID[s], D]
            blk = S.T.reshape(NCH, 128, WID[s]).transpose(1, 0, 2)
            h = 0 if s < 3 else 1
            o = SOFF_X[s]
            xt_host[g * 2 + h, :, o:o + NCH * WID[s]] = \
                blk.reshape(128, NCH * WID[s]).astype(BF16)
    return xt_host


def _unshard_core(o):
    # o: [D, RC] cols ordered (g, t, w); group g == local batch row,
    # window w at in-window position t -> time w*10+t
    return np.ascontiguousarray(
        o.reshape(D, NG, W, GWN).transpose(1, 3, 2, 0).reshape(BC, T, D))


_NC_CACHE = None


def _prep_in_maps(x, conv_w, conv_b, gate_w, gate_b):
    cw_host, gw_host, cb_host, gb_host = _prep_weights(
        conv_w, conv_b, gate_w, gate_b)
    return [
        {"xt": _prep_core_x(x[BC * i:BC * (i + 1)]), "cwr": cw_host,
         "gwr": gw_host, "cb": cb_host, "gb": gb_host}
        for i in range(NCORES)
    ]


def kernel(x, conv_w, conv_b, gate_w, gate_b):
    global _NC_CACHE
    x = np.asarray(x, np.float32)
    conv_w = np.asarray(conv_w, np.float32)
    conv_b = np.asarray(conv_b, np.float32)
    gate_w = np.asarray(gate_w, np.float32)
    gate_b = np.asarray(gate_b, np.float32)

    in_maps = _prep_in_maps(x, conv_w, conv_b, gate_w, gate_b)
    if _NC_CACHE is None:
        _NC_CACHE = _build()
    res = run_bass_kernel_spmd(_NC_CACHE, in_maps,
                               core_ids=list(range(NCORES))).results

    out = np.empty((B, T, D), np.float32)
    for i in range(NCORES):
        out[BC * i:BC * (i + 1)] = _unshard_core(np.asarray(res[i]["outT"]))
    return out


# revision 28
# speedup vs baseline: 1.0280x; 1.0099x over previous
"""Trainium2 Bass kernel for windowed Conv1d(k=3) + sigmoid gating.

Reference (B=16, T=960, D=1024, W=10): windows of 10 conv'd independently
with per-window zero pad 1:
    cnn[t, d] = sum_{k,c} conv_w[d, c, k] * xpad[t + k, c]
    out = cnn * sigmoid(cnn @ gate_w.T + gate_b)

Strategy: data parallel over 8 cores (2 batches / 192 windows / core).
Hybrid Winograd conv: two F(4,3) tiles (outputs 0-3 from xp[0:6], outputs
4-7 from xp[4:10]) at points {0,1,-1,2,-1/2,inf} plus an F(2,3) tail
(outputs 8,9 from xp[8:12]) at points {0,1,-1,inf} folded into the same
weight streams => 16 muls/window (vs 30 direct).  Streams use canonical
Vandermonde weights g_b = [1,b,b^2].W (normalization in the host-side
V^{-T} input transforms).

Per core: 2 groups of 96 windows (group == local batch row).  Per
(group, dck): 6 accumulation chains into one 4-bank PSUM slot packed
{m1+m3 | m2+m4 | m0 | m5} (chains may not cross banks).  The combine is
copy-first: ACT copies each bank to bf16 SBUF as its chains finish
(frees the slot fast => no PE stall at unit boundaries), then the A^T
combine runs in bf16 on DVE/GpSimd/ACT with ~1.6us/unit slack per engine.
Tails use a single class (no rotation): stream columns are [A|B|T].

Phases: g0 convs -> all g0 gates -> g1 convs -> g1 gates; xt[g1]
reloads into the xt slots during the g0 gate phase (25us window);
gwr is ordered after cwr in the DMA stream (first-use order).  A short
cold warmup (~12 matmuls) covers the preamble->first-data window.
"""

import numpy as np
import ml_dtypes

import concourse.bacc as bacc
import concourse.bass as bass
import concourse.tile as tile
from concourse import mybir
from concourse.bass_utils import run_bass_kernel_spmd

BF16 = ml_dtypes.bfloat16

B, T, D, W = 16, 960, 1024, 10
NCORES = 8
BC = B // NCORES             # batches per core (2)
NWIN = BC * T // W           # windows per core (192)
RC = NWIN * W                # output rows per core (1920)
NG = 2                       # groups per core (group == local batch row)
GWN = NWIN // NG             # windows per group (96)
GN = GWN * W                 # output cols per group (960)
NCH = D // 128               # channel chunks (8)
AF = mybir.ActivationFunctionType

INF = "inf"
PTS = [0.0, 1.0, -1.0, 2.0, -0.5, INF]          # logical nodes
TPTS = [0.0, 1.0, -1.0, INF]                     # tail nodes
# storage (chain) order of streams: nodes per storage slot
SNODE = [1.0, 2.0, -1.0, -0.5, 0.0, INF]         # m1 m3 m2 m4 m0 m5
SW = [3, 2, 3, 2, 3, 3]                          # width in 96-col blocks
WID = [w * GWN for w in SW]                      # [288,192,288,192,288,288]
# tail row in TPTS for storage streams that carry a tail block
TROW = {0: 1, 2: 2, 4: 0, 5: 3}
# per-half storage streams (matches xt half layout and cwr layout)
HSTREAMS = [(0, 1, 2), (3, 4, 5)]
HCOLS = 6144                                     # cols per xt half (768*8)
# psum col offset per storage stream (4 banks of 512 fp32)
POFF = [0, 288, 512, 800, 1024, 1536]
# bf16 mS col offset per storage stream (packed, 1536 cols)
MOFF = [0, 288, 480, 768, 960, 1248]
# cwr col offset of (storage stream, ck) block
SOFF_X = []                                      # xt col offset per stream
_off = 0
for _s in range(6):
    SOFF_X.append(_off if _s < 3 else _off - HCOLS)
    _off += WID[_s] * NCH
AB = 2 * GWN                                     # 192 (A|B block)
TL, TH = AB, AB + GWN                            # tail block 192:288


def _vinv_T(points):
    n = len(points)
    V = np.zeros((n, n))
    for j, b in enumerate(points):
        if b is INF:
            V[j, n - 1] = 1.0
        else:
            V[j] = [float(b) ** i for i in range(n)]
    return np.linalg.inv(V).T


BA = _vinv_T(PTS)
BT4 = _vinv_T(TPTS)
LOGI = [PTS.index(b) for b in SNODE]             # storage -> logical row


def _build():
    nc = bacc.Bacc("TRN2", target_bir_lowering=False, debug=False)

    # xt[g*2+h]: [cc, (storage stream, ck, col)] transformed input halves
    xt = nc.dram_tensor("xt", [NG * 2, 128, HCOLS], mybir.dt.bfloat16,
                        kind="ExternalInput")
    # cwr[dck]: [cc, (s*8+ck)*128+dd] conv lhsT blocks, storage order
    cwr = nc.dram_tensor("cwr", [NCH, 128, 6 * NCH * 128], mybir.dt.bfloat16,
                         kind="ExternalInput")
    # gwr[eck]: [dd, dck*128+ee] gate lhsT blocks
    gwr = nc.dram_tensor("gwr", [NCH, 128, NCH * 128], mybir.dt.bfloat16,
                         kind="ExternalInput")
    cb = nc.dram_tensor("cb", [128, NCH], mybir.dt.float32, kind="ExternalInput")
    gb = nc.dram_tensor("gb", [128, NCH], mybir.dt.float32, kind="ExternalInput")
    outT = nc.dram_tensor("outT", [D, RC], mybir.dt.float32, kind="ExternalOutput")

    with tile.TileContext(nc) as tc:
        with (
            tc.tile_pool(name="consts", bufs=1) as consts,
            tc.tile_pool(name="xtp", bufs=1) as xtp,
            tc.tile_pool(name="cnn", bufs=1) as cnnp,
            tc.tile_pool(name="ms", bufs=2) as msp,
            tc.tile_pool(name="work", bufs=2) as work,
            tc.tile_pool(name="gout", bufs=4) as gout,
            tc.tile_pool(name="psum", bufs=2, space="PSUM") as psum,
        ):
            cwr_sb = [None] * NCH
            xt_sb = [None, None]

            def load_xt_half(g, h):
                # scalar (ACT) HWDGE queue: parallel to the sync queue
                t = xtp.tile([128, HCOLS], mybir.dt.bfloat16, tag=f"xt{h}",
                             name=f"xt{h}_g{g}")
                nc.scalar.dma_start(t[:], xt[g * 2 + h])
                xt_sb[h] = t

            # ---- DMA ramp (first-use order, per-stream pieces for dck0) ----
            cw0 = consts.tile([128, 6 * NCH * 128], mybir.dt.bfloat16, tag="cw0")
            cwr_sb[0] = cw0
            xt_sb[0] = xtp.tile([128, HCOLS], mybir.dt.bfloat16, tag="xt0",
                                name="xt0_g0")
            xt_sb[1] = xtp.tile([128, HCOLS], mybir.dt.bfloat16, tag="xt1",
                                name="xt1_g0")
            # dual-queue ramp: cw0 stream pieces + cwr1-7 + gwr on the sync
            # (SP) queue; xt pieces + cb/gb on the scalar (ACT) queue.  Each
            # DMA instruction costs ~650ns of queue issue time, so splitting
            # halves the serial issue latency of the ramp.
            SB = NCH * 128                       # cwr cols per stream (1024)
            for s in range(6):
                h = 0 if s < 3 else 1
                nc.sync.dma_start(cw0[:, s * SB:(s + 1) * SB],
                                  cwr[0][:, s * SB:(s + 1) * SB])
                o = SOFF_X[s]
                nc.sync.dma_start(xt_sb[h][:, o:o + NCH * WID[s]],
                                  xt[h][:, o:o + NCH * WID[s]])
            # tiny bias loads on the scalar queue (off the critical sync FIFO)
            cb_sb = consts.tile([128, NCH], mybir.dt.float32, tag="cb")
            nc.scalar.dma_start(cb_sb[:], cb[:])
            gb_sb = consts.tile([128, NCH], mybir.dt.float32, tag="gb")
            nc.scalar.dma_start(gb_sb[:], gb[:])
            for dck in range(1, NCH):
                cwt = consts.tile([128, 6 * NCH * 128], mybir.dt.bfloat16,
                                  tag=f"cw{dck}")
                cwr_sb[dck] = cwt
                nc.sync.dma_start(cwt[:], cwr[dck])
            gwr_sb = []
            for eck in range(NCH):
                t = consts.tile([128, NCH * 128], mybir.dt.bfloat16, tag=f"gw{eck}")
                nc.sync.dma_start(t[:], gwr[eck])
                gwr_sb.append(t)

            # ---- HAM warmup over the preamble->first-data window ----
            scr = consts.tile([128, 256], mybir.dt.bfloat16, tag="scr")
            nc.gpsimd.memset(scr[:], 0.0)
            # enough cold matmuls (~5.5us) to bridge clock-start -> first
            # conv data; keeps the HAM busy-window unbroken so real chains
            # run at 8/8 from the start
            wps = psum.tile([128, 2048], mybir.dt.float32, tag="cps")
            for _ in range(22):
                nc.tensor.matmul(wps[:, :256], scr[:, :128], scr[:, :256],
                                 start=True, stop=True)

            cnn_t = [[None] * NCH for _ in range(NG)]

            def conv_unit(g, dck, pad_intra=False, pad_end=0):
                ps = psum.tile([128, 2048], mybir.dt.float32, tag="cps")
                for s in range(6):
                    h = 0 if s < 3 else 1
                    w = WID[s]
                    for ck in range(NCH):
                        nc.tensor.matmul(
                            ps[:, POFF[s]:POFF[s] + w],
                            cwr_sb[dck][:, (s * NCH + ck) * 128:
                                        (s * NCH + ck + 1) * 128],
                            xt_sb[h][:, SOFF_X[s] + ck * w:
                                     SOFF_X[s] + (ck + 1) * w],
                            start=(ck == 0),
                            stop=(ck == NCH - 1),
                        )
                    # anti-idle fillers into this slot's unused pad cols
                    # (bank3 tail): keep HAM at 8/8 while the ramp DMA
                    # trickles chain inputs in; never read.
                    npad = (3 if pad_intra else 0) if s < 5 else pad_end
                    for _ in range(npad):
                        nc.tensor.matmul(ps[:, 1824:2048], scr[:, :128],
                                         scr[:, :224], start=True, stop=True)
                combine(g, dck, ps)

            def combine(g, dck, ps):
                cbs = cb_sb[:, dck:dck + 1]
                bf = mybir.dt.bfloat16
                mS = msp.tile([128, 1536], bf, tag="ms")

                # copy-first: one ACT copy per psum bank, in chain order;
                # the slot frees after c4 (~1us after the unit's last MM)
                nc.scalar.activation(mS[:, 0:480], ps[:, 0:480], AF.Copy)
                nc.scalar.activation(mS[:, 480:960], ps[:, 512:992], AF.Copy)
                nc.scalar.activation(mS[:, 960:1248], ps[:, 1024:1312], AF.Copy)
                nc.scalar.activation(mS[:, 1248:1536], ps[:, 1536:1824], AF.Copy)
                m1 = mS[:, MOFF[0]:MOFF[0] + 288]
                m3 = mS[:, MOFF[1]:MOFF[1] + 192]
                m2 = mS[:, MOFF[2]:MOFF[2] + 288]
                m4 = mS[:, MOFF[3]:MOFF[3] + 192]
                m0 = mS[:, MOFF[4]:MOFF[4] + 288]
                m5 = mS[:, MOFF[5]:MOFF[5] + 288]

                def st(tag, n):
                    return work.tile([128, n], bf, tag=tag, name=tag)

                cnn = cnnp.tile([128, GN], bf, tag=f"cnn{g}_{dck}")
                cnn_t[g][dck] = cnn
                v = cnn[:].rearrange("p (t w) -> p t w", w=GWN)

                def outab(t0):
                    # A-tile output t0 and B-tile output t0+4: [128, 2, 96]
                    return v[:, t0:t0 + 5:4]

                # u_k = b^k m3 + cb on ACT; h_k = (-1/2)^k m4 on DVE
                u1 = st("u1", AB)
                nc.scalar.activation(u1[:], m3, AF.Identity, bias=cbs, scale=2.0)
                u2 = st("u2", AB)
                nc.scalar.activation(u2[:], m3, AF.Identity, bias=cbs, scale=4.0)
                u3 = st("u3", AB)
                nc.scalar.activation(u3[:], m3, AF.Identity, bias=cbs, scale=8.0)
                s1 = st("s1", 288); nc.vector.tensor_add(s1[:], m1, m2)
                d1 = st("d1", 288); nc.vector.tensor_sub(d1[:], m1, m2)
                h1 = st("h1", AB); nc.vector.tensor_scalar_mul(h1[:], m4, -0.5)
                h2 = st("h2", AB); nc.vector.tensor_scalar_mul(h2[:], m4, 0.25)
                h3 = st("h3", AB); nc.vector.tensor_scalar_mul(h3[:], m4, -0.125)
                a0 = st("a0", AB); nc.vector.tensor_add(a0[:], m3, m4)
                t0 = st("t0", AB); nc.vector.tensor_scalar_add(t0[:], a0[:], cbs)
                P = st("P", 288)
                nc.gpsimd.tensor_add(P[:], m0, s1[:])
                nc.gpsimd.tensor_add(outab(0), P[:, :AB], t0[:])          # y0
                w1 = st("w1", AB); nc.gpsimd.tensor_add(w1[:], d1[:, :AB], u1[:])
                nc.gpsimd.tensor_add(outab(1), w1[:], h1[:])              # y1
                w2 = st("w2", AB); nc.gpsimd.tensor_add(w2[:], s1[:, :AB], u2[:])
                nc.gpsimd.tensor_add(outab(2), w2[:], h2[:])              # y2
                w3 = st("w3", AB); nc.vector.tensor_add(w3[:], d1[:, :AB], u3[:])
                x3 = st("x3", AB); nc.vector.tensor_add(x3[:], w3[:], h3[:])
                nc.gpsimd.tensor_add(outab(3), x3[:], m5[:, :AB])         # y3
                # tails: y8 = P[T]+cb ; y9 = d1[T] + m5[T] + cb
                nc.vector.tensor_scalar_add(v[:, 8, :], P[:, TL:TH], cbs)
                e9 = st("e9", GWN)
                nc.vector.tensor_add(e9[:], d1[:, TL:TH], m5[:, TL:TH])
                nc.vector.tensor_scalar_add(v[:, 9, :], e9[:], cbs)

            def gate_quad(g, eck0, tail=False):
                # 4 chains (eck0,c0),(eck0,c1),(eck0+1,c0),(eck0+1,c1) into
                # banks 0-3 of one psum slot, accumulated dck-MAJOR: the
                # cnn[dck=7]-dependent matmuls land ~5.7us of PE work after
                # a conv phase end (covers the trailing combine backlog),
                # and the deep chain pipeline avoids psum-WAR stalls.
                # tail=True (very last quad): chain 3 runs alone after the
                # other three so only its short split epilog trails the
                # kernel's last matmul.
                chains = [(eck0, 0), (eck0, 1), (eck0 + 1, 0), (eck0 + 1, 1)]

                def epilog(pst, q, eck, c, split=False):
                    gt = gout.tile([128, 480], mybir.dt.bfloat16, tag="gate")
                    ot = gout.tile([128, 480], mybir.dt.float32, tag="out")
                    chunks = ((0, 240), (240, 480)) if split else ((0, 480),)
                    for lo, hi in chunks:
                        nc.scalar.activation(gt[:, lo:hi],
                                             pst[:, q * 512 + lo:q * 512 + hi],
                                             AF.Sigmoid,
                                             bias=gb_sb[:, eck:eck + 1])
                        nc.vector.tensor_mul(ot[:, lo:hi],
                                             cnn_t[g][eck][:, c * 480 + lo:
                                                           c * 480 + hi],
                                             gt[:, lo:hi])
                        # the kernel's very last pieces issue from the (then
                        # idle) scalar queue so they don't queue behind the
                        # sync FIFO's earlier out-DMA issues
                        eng = nc.scalar if split else nc.sync
                        eng.dma_start(
                            outT[eck * 128:(eck + 1) * 128,
                                 g * GN + c * 480 + lo:g * GN + c * 480 + hi],
                            ot[:, lo:hi])

                if tail:
                    # chains 0-2 dck-major in ps2; chain 3 in its OWN
                    # rotated tile with matmuls emitted BEFORE the other
                    # epilogs (tile-granular WAR tracking would stall them
                    # on the sigmoids reading ps2), so only chain 3's short
                    # split epilog trails the kernel's final matmul.
                    ps2 = psum.tile([128, 2048], mybir.dt.float32, tag="cps")
                    for dck in range(NCH):
                        for q in range(3):
                            eck, c = chains[q]
                            nc.tensor.matmul(
                                ps2[:, q * 512:q * 512 + 480],
                                gwr_sb[eck][:, dck * 128:(dck + 1) * 128],
                                cnn_t[g][dck][:, c * 480:(c + 1) * 480],
                                start=(dck == 0),
                                stop=(dck == NCH - 1),
                            )
                    ps3 = psum.tile([128, 2048], mybir.dt.float32, tag="cps",
                                    name="tailq3")
                    eck3, c3 = chains[3]
                    for dck in range(NCH):
                        nc.tensor.matmul(
                            ps3[:, :480],
                            gwr_sb[eck3][:, dck * 128:(dck + 1) * 128],
                            cnn_t[g][dck][:, c3 * 480:(c3 + 1) * 480],
                            start=(dck == 0),
                            stop=(dck == NCH - 1),
                        )
                    for q in range(3):
                        epilog(ps2, q, *chains[q])
                    epilog(ps3, 0, eck3, c3, split=True)
                else:
                    ps2 = psum.tile([128, 2048], mybir.dt.float32, tag="cps")
                    for dck in range(NCH):
                        for q, (eck, c) in enumerate(chains):
                            nc.tensor.matmul(
                                ps2[:, q * 512:q * 512 + 480],
                                gwr_sb[eck][:, dck * 128:(dck + 1) * 128],
                                cnn_t[g][dck][:, c * 480:(c + 1) * 480],
                                start=(dck == 0),
                                stop=(dck == NCH - 1),
                            )
                    for q, (eck, c) in enumerate(chains):
                        epilog(ps2, q, eck, c)

            # phase g0 convs (cwr streams in behind; early units pad their
            # DMA-trickle idle with anti-idle matmuls)
            for dck in range(NCH):
                conv_unit(0, dck, pad_intra=(dck < 2),
                          pad_end=(4 if dck < 4 else 0))
            # xt g1 reloads during the g0 gate phase (WAR on the xt slots
            # releases at g0-conv end; the 25us gate window covers it)
            load_xt_half(1, 0)
            load_xt_half(1, 1)
            for eck0 in range(0, NCH, 2):
                gate_quad(0, eck0)
            for dck in range(NCH):
                conv_unit(1, dck)
            for eck0 in range(0, NCH, 2):
                gate_quad(1, eck0, tail=(eck0 == NCH - 2))
    nc.compile()
    return nc


def _prep_weights(conv_w, conv_b, gate_w, gate_b):
    W0, W1, W2 = [conv_w[:, :, k].astype(np.float64) for k in range(3)]
    g = []
    for b in SNODE:
        g.append(W2 if b is INF else W0 + b * W1 + b * b * W2)
    garr = np.stack(g)                                   # [6 storage, Do, Di]
    gv = garr.reshape(6, NCH, 128, NCH, 128)             # [s, dck, dd, ck, cc]
    cw_host = np.ascontiguousarray(gv.transpose(1, 4, 0, 3, 2)).reshape(
        NCH, 128, 6 * NCH * 128).astype(BF16)
    gwt = gate_w.T.reshape(NCH, 128, NCH, 128)           # [dck, dd, eck, ee]
    gw_host = np.ascontiguousarray(gwt.transpose(2, 1, 0, 3)).reshape(
        NCH, 128, NCH * 128).astype(BF16)
    cb_host = np.ascontiguousarray(conv_b.reshape(NCH, 128).T).astype(np.float32)
    gb_host = np.ascontiguousarray(gate_b.reshape(NCH, 128).T).astype(np.float32)
    return cw_host, gw_host, cb_host, gb_host


def _prep_core_x(x_shard):
    # x_shard [BC, T, D] -> xt [NG*2, 128, HCOLS]
    xw = x_shard.reshape(NWIN, W, D).astype(np.float64)
    xp = np.pad(xw, ((0, 0), (1, 1), (0, 0)))            # [192, 12, D]
    xt_host = np.empty((NG * 2, 128, HCOLS), BF16)
    for g in range(NG):
        ws = xp[g * GWN:(g + 1) * GWN]                   # [96, 12, D]
        xA = np.einsum('ji,wic->jwc', BA, ws[:, 0:6])    # [6, 96, D]
        xB = np.einsum('ji,wic->jwc', BA, ws[:, 4:10])
        xT = np.einsum('ji,wic->jwc', BT4, ws[:, 8:12])  # [4, 96, D]
        for s in range(6):
            j = LOGI[s]
            parts = [xA[j], xB[j]]
            if s in TROW:
                parts.append(xT[TROW[s]])
            S = np.concatenate(parts, axis=0)            # [WID[s], D]
            blk = S.T.reshape(NCH, 128, WID[s]).transpose(1, 0, 2)
            h = 0 if s < 3 else 1
            o = SOFF_X[s]
            xt_host[g * 2 + h, :, o:o + NCH * WID[s]] = \
                blk.reshape(128, NCH * WID[s]).astype(BF16)
    return xt_host


def _unshard_core(o):
    # o: [D, RC] cols ordered (g, t, w); group g == local batch row,
    # window w at in-window position t -> time w*10+t
    return np.ascontiguousarray(
        o.reshape(D, NG, W, GWN).transpose(1, 3, 2, 0).reshape(BC, T, D))


_NC_CACHE = None


def _prep_in_maps(x, conv_w, conv_b, gate_w, gate_b):
    cw_host, gw_host, cb_host, gb_host = _prep_weights(
        conv_w, conv_b, gate_w, gate_b)
    return [
        {"xt": _prep_core_x(x[BC * i:BC * (i + 1)]), "cwr": cw_host,
         "gwr": gw_host, "cb": cb_host, "gb": gb_host}
        for i in range(NCORES)
    ]


def kernel(x, conv_w, conv_b, gate_w, gate_b):
    global _NC_CACHE
    x = np.asarray(x, np.float32)
    conv_w = np.asarray(conv_w, np.float32)
    conv_b = np.asarray(conv_b, np.float32)
    gate_w = np.asarray(gate_w, np.float32)
    gate_b = np.asarray(gate_b, np.float32)

    in_maps = _prep_in_maps(x, conv_w, conv_b, gate_w, gate_b)
    if _NC_CACHE is None:
        _NC_CACHE = _build()
    res = run_bass_kernel_spmd(_NC_CACHE, in_maps,
                               core_ids=list(range(NCORES))).results

    out = np.empty((B, T, D), np.float32)
    for i in range(NCORES):
        out[BC * i:BC * (i + 1)] = _unshard_core(np.asarray(res[i]["outT"]))
    return out


# revision 29
# speedup vs baseline: 1.0411x; 1.0127x over previous
"""Trainium2 Bass kernel for windowed Conv1d(k=3) + sigmoid gating.

Reference (B=16, T=960, D=1024, W=10): windows of 10 conv'd independently
with per-window zero pad 1:
    cnn[t, d] = sum_{k,c} conv_w[d, c, k] * xpad[t + k, c]
    out = cnn * sigmoid(cnn @ gate_w.T + gate_b)

Strategy: data parallel over 8 cores (2 batches / 192 windows / core).
Hybrid Winograd conv: two F(4,3) tiles (outputs 0-3 from xp[0:6], outputs
4-7 from xp[4:10]) at points {0,1,-1,2,-1/2,inf} plus an F(2,3) tail
(outputs 8,9 from xp[8:12]) at points {0,1,-1,inf} folded into the same
weight streams => 16 muls/window (vs 30 direct).  Streams use canonical
Vandermonde weights g_b = [1,b,b^2].W (normalization in the host-side
V^{-T} input transforms).

Per core: 2 groups of 96 windows (group == local batch row).  Per
(group, dck): 6 accumulation chains into one 4-bank PSUM slot packed
{m1+m3 | m2+m4 | m0 | m5} (chains may not cross banks).  The combine is
copy-first: ACT copies each bank to bf16 SBUF as its chains finish
(frees the slot fast => no PE stall at unit boundaries), then the A^T
combine runs in bf16 on DVE/GpSimd/ACT with ~1.6us/unit slack per engine.
Tails use a single class (no rotation): stream columns are [A|B|T].

Phases: g0 convs -> all g0 gates -> g1 convs -> g1 gates; xt[g1]
reloads into the xt slots during the g0 gate phase (25us window);
gwr is ordered after cwr in the DMA stream (first-use order).  A short
cold warmup (~12 matmuls) covers the preamble->first-data window.
"""

import numpy as np
import ml_dtypes

import concourse.bacc as bacc
import concourse.bass as bass
import concourse.tile as tile
from concourse import mybir
from concourse.bass_utils import run_bass_kernel_spmd

BF16 = ml_dtypes.bfloat16

B, T, D, W = 16, 960, 1024, 10
NCORES = 8
BC = B // NCORES             # batches per core (2)
NWIN = BC * T // W           # windows per core (192)
RC = NWIN * W                # output rows per core (1920)
NG = 2                       # groups per core (group == local batch row)
GWN = NWIN // NG             # windows per group (96)
GN = GWN * W                 # output cols per group (960)
NCH = D // 128               # channel chunks (8)
AF = mybir.ActivationFunctionType

INF = "inf"
PTS = [0.0, 1.0, -1.0, 2.0, -0.5, INF]          # logical nodes
TPTS = [0.0, 1.0, -1.0, INF]                     # tail nodes
# storage (chain) order of streams: nodes per storage slot
SNODE = [1.0, 2.0, -1.0, -0.5, 0.0, INF]         # m1 m3 m2 m4 m0 m5
SW = [3, 2, 3, 2, 3, 3]                          # width in 96-col blocks
WID = [w * GWN for w in SW]                      # [288,192,288,192,288,288]
# tail row in TPTS for storage streams that carry a tail block
TROW = {0: 1, 2: 2, 4: 0, 5: 3}
# per-half storage streams (matches xt half layout and cwr layout)
HSTREAMS = [(0, 1, 2), (3, 4, 5)]
HCOLS = 6144                                     # cols per xt half (768*8)
# psum col offset per storage stream (4 banks of 512 fp32)
POFF = [0, 288, 512, 800, 1024, 1536]
# bf16 mS col offset per storage stream (packed, 1536 cols)
MOFF = [0, 288, 480, 768, 960, 1248]
# cwr col offset of (storage stream, ck) block
SOFF_X = []                                      # xt col offset per stream
_off = 0
for _s in range(6):
    SOFF_X.append(_off if _s < 3 else _off - HCOLS)
    _off += WID[_s] * NCH
AB = 2 * GWN                                     # 192 (A|B block)
TL, TH = AB, AB + GWN                            # tail block 192:288


def _vinv_T(points):
    n = len(points)
    V = np.zeros((n, n))
    for j, b in enumerate(points):
        if b is INF:
            V[j, n - 1] = 1.0
        else:
            V[j] = [float(b) ** i for i in range(n)]
    return np.linalg.inv(V).T


BA = _vinv_T(PTS)
BT4 = _vinv_T(TPTS)
LOGI = [PTS.index(b) for b in SNODE]             # storage -> logical row


def _build():
    nc = bacc.Bacc("TRN2", target_bir_lowering=False, debug=False)

    # xt[g*2+h]: [cc, (storage stream, ck, col)] transformed input halves
    xt = nc.dram_tensor("xt", [NG * 2, 128, HCOLS], mybir.dt.bfloat16,
                        kind="ExternalInput")
    # cwr[dck]: [cc, (s*8+ck)*128+dd] conv lhsT blocks, storage order
    cwr = nc.dram_tensor("cwr", [NCH, 128, 6 * NCH * 128], mybir.dt.bfloat16,
                         kind="ExternalInput")
    # gwr[eck]: [dd, dck*128+ee] gate lhsT blocks
    gwr = nc.dram_tensor("gwr", [NCH, 128, NCH * 128], mybir.dt.bfloat16,
                         kind="ExternalInput")
    cb = nc.dram_tensor("cb", [128, NCH], mybir.dt.float32, kind="ExternalInput")
    gb = nc.dram_tensor("gb", [128, NCH], mybir.dt.float32, kind="ExternalInput")
    outT = nc.dram_tensor("outT", [D, RC], mybir.dt.float32, kind="ExternalOutput")

    with tile.TileContext(nc) as tc:
        with (
            tc.tile_pool(name="consts", bufs=1) as consts,
            tc.tile_pool(name="xtp", bufs=1) as xtp,
            tc.tile_pool(name="cnn", bufs=1) as cnnp,
            tc.tile_pool(name="ms", bufs=2) as msp,
            tc.tile_pool(name="work", bufs=2) as work,
            tc.tile_pool(name="gout", bufs=4) as gout,
            tc.tile_pool(name="psum", bufs=2, space="PSUM") as psum,
        ):
            cwr_sb = [None] * NCH
            xt_sb = [None, None]

            def load_xt_half(g, h):
                # scalar (ACT) HWDGE queue: parallel to the sync queue
                t = xtp.tile([128, HCOLS], mybir.dt.bfloat16, tag=f"xt{h}",
                             name=f"xt{h}_g{g}")
                nc.scalar.dma_start(t[:], xt[g * 2 + h])
                xt_sb[h] = t

            # ---- DMA ramp (first-use order, per-stream pieces for dck0) ----
            cw0 = consts.tile([128, 6 * NCH * 128], mybir.dt.bfloat16, tag="cw0")
            cwr_sb[0] = cw0
            xt_sb[0] = xtp.tile([128, HCOLS], mybir.dt.bfloat16, tag="xt0",
                                name="xt0_g0")
            xt_sb[1] = xtp.tile([128, HCOLS], mybir.dt.bfloat16, tag="xt1",
                                name="xt1_g0")
            # dual-queue ramp: cw0 stream pieces + cwr1-7 + gwr on the sync
            # (SP) queue; xt pieces + cb/gb on the scalar (ACT) queue.  Each
            # DMA instruction costs ~650ns of queue issue time, so splitting
            # halves the serial issue latency of the ramp.
            SB = NCH * 128                       # cwr cols per stream (1024)
            for s in range(6):
                h = 0 if s < 3 else 1
                nc.sync.dma_start(cw0[:, s * SB:(s + 1) * SB],
                                  cwr[0][:, s * SB:(s + 1) * SB])
                o = SOFF_X[s]
                nc.sync.dma_start(xt_sb[h][:, o:o + NCH * WID[s]],
                                  xt[h][:, o:o + NCH * WID[s]])
            # tiny bias loads on the scalar queue (off the critical sync FIFO)
            cb_sb = consts.tile([128, NCH], mybir.dt.float32, tag="cb")
            nc.scalar.dma_start(cb_sb[:], cb[:])
            gb_sb = consts.tile([128, NCH], mybir.dt.float32, tag="gb")
            nc.scalar.dma_start(gb_sb[:], gb[:])
            HB = 3 * NCH * 128
            for dck in range(1, NCH):
                cwt = consts.tile([128, 6 * NCH * 128], mybir.dt.bfloat16,
                                  tag=f"cw{dck}")
                cwr_sb[dck] = cwt
                if dck < 5:
                    # half-tile pieces: the unit's first three chains only
                    # need h0, so it starts ~2us before the full tile lands
                    nc.sync.dma_start(cwt[:, :HB], cwr[dck][:, :HB])
                    nc.sync.dma_start(cwt[:, HB:], cwr[dck][:, HB:])
                else:
                    nc.sync.dma_start(cwt[:], cwr[dck])
            gwr_sb = []
            for eck in range(NCH):
                t = consts.tile([128, NCH * 128], mybir.dt.bfloat16, tag=f"gw{eck}")
                nc.sync.dma_start(t[:], gwr[eck])
                gwr_sb.append(t)

            # ---- HAM warmup over the preamble->first-data window ----
            scr = consts.tile([128, 256], mybir.dt.bfloat16, tag="scr")
            nc.gpsimd.memset(scr[:], 0.0)
            # enough cold matmuls (~5.5us) to bridge clock-start -> first
            # conv data; keeps the HAM busy-window unbroken so real chains
            # run at 8/8 from the start
            wps = psum.tile([128, 2048], mybir.dt.float32, tag="cps")
            for _ in range(22):
                nc.tensor.matmul(wps[:, :256], scr[:, :128], scr[:, :256],
                                 start=True, stop=True)

            cnn_t = [[None] * NCH for _ in range(NG)]

            def conv_unit(g, dck, pad_intra=False, pad_end=0):
                ps = psum.tile([128, 2048], mybir.dt.float32, tag="cps")
                for s in range(6):
                    h = 0 if s < 3 else 1
                    w = WID[s]
                    for ck in range(NCH):
                        nc.tensor.matmul(
                            ps[:, POFF[s]:POFF[s] + w],
                            cwr_sb[dck][:, (s * NCH + ck) * 128:
                                        (s * NCH + ck + 1) * 128],
                            xt_sb[h][:, SOFF_X[s] + ck * w:
                                     SOFF_X[s] + (ck + 1) * w],
                            start=(ck == 0),
                            stop=(ck == NCH - 1),
                        )
                    # anti-idle fillers into this slot's unused pad cols
                    # (bank3 tail): keep HAM at 8/8 while the ramp DMA
                    # trickles chain inputs in; never read.
                    npad = (3 if pad_intra else 0) if s < 5 else pad_end
                    for _ in range(npad):
                        nc.tensor.matmul(ps[:, 1824:2048], scr[:, :128],
                                         scr[:, :224], start=True, stop=True)
                combine(g, dck, ps)

            def combine(g, dck, ps):
                cbs = cb_sb[:, dck:dck + 1]
                bf = mybir.dt.bfloat16
                mS = msp.tile([128, 1536], bf, tag="ms")

                # copy-first: one ACT copy per psum bank, in chain order;
                # the slot frees after c4 (~1us after the unit's last MM)
                nc.scalar.activation(mS[:, 0:480], ps[:, 0:480], AF.Copy)
                nc.scalar.activation(mS[:, 480:960], ps[:, 512:992], AF.Copy)
                nc.scalar.activation(mS[:, 960:1248], ps[:, 1024:1312], AF.Copy)
                nc.scalar.activation(mS[:, 1248:1536], ps[:, 1536:1824], AF.Copy)
                m1 = mS[:, MOFF[0]:MOFF[0] + 288]
                m3 = mS[:, MOFF[1]:MOFF[1] + 192]
                m2 = mS[:, MOFF[2]:MOFF[2] + 288]
                m4 = mS[:, MOFF[3]:MOFF[3] + 192]
                m0 = mS[:, MOFF[4]:MOFF[4] + 288]
                m5 = mS[:, MOFF[5]:MOFF[5] + 288]

                def st(tag, n):
                    return work.tile([128, n], bf, tag=tag, name=tag)

                cnn = cnnp.tile([128, GN], bf, tag=f"cnn{g}_{dck}")
                cnn_t[g][dck] = cnn
                v = cnn[:].rearrange("p (t w) -> p t w", w=GWN)

                def outab(t0):
                    # A-tile output t0 and B-tile output t0+4: [128, 2, 96]
                    return v[:, t0:t0 + 5:4]

                # u_k = b^k m3 + cb on ACT; h_k = (-1/2)^k m4 on DVE
                u1 = st("u1", AB)
                nc.scalar.activation(u1[:], m3, AF.Identity, bias=cbs, scale=2.0)
                u2 = st("u2", AB)
                nc.scalar.activation(u2[:], m3, AF.Identity, bias=cbs, scale=4.0)
                u3 = st("u3", AB)
                nc.scalar.activation(u3[:], m3, AF.Identity, bias=cbs, scale=8.0)
                s1 = st("s1", 288); nc.vector.tensor_add(s1[:], m1, m2)
                d1 = st("d1", 288); nc.vector.tensor_sub(d1[:], m1, m2)
                h1 = st("h1", AB); nc.vector.tensor_scalar_mul(h1[:], m4, -0.5)
                h2 = st("h2", AB); nc.vector.tensor_scalar_mul(h2[:], m4, 0.25)
                h3 = st("h3", AB); nc.vector.tensor_scalar_mul(h3[:], m4, -0.125)
                a0 = st("a0", AB); nc.vector.tensor_add(a0[:], m3, m4)
                t0 = st("t0", AB); nc.vector.tensor_scalar_add(t0[:], a0[:], cbs)
                P = st("P", 288)
                nc.gpsimd.tensor_add(P[:], m0, s1[:])
                nc.gpsimd.tensor_add(outab(0), P[:, :AB], t0[:])          # y0
                w1 = st("w1", AB); nc.gpsimd.tensor_add(w1[:], d1[:, :AB], u1[:])
                nc.gpsimd.tensor_add(outab(1), w1[:], h1[:])              # y1
                w2 = st("w2", AB); nc.gpsimd.tensor_add(w2[:], s1[:, :AB], u2[:])
                nc.gpsimd.tensor_add(outab(2), w2[:], h2[:])              # y2
                w3 = st("w3", AB); nc.vector.tensor_add(w3[:], d1[:, :AB], u3[:])
                x3 = st("x3", AB); nc.vector.tensor_add(x3[:], w3[:], h3[:])
                nc.gpsimd.tensor_add(outab(3), x3[:], m5[:, :AB])         # y3
                # tails: y8 = P[T]+cb ; y9 = d1[T] + m5[T] + cb
                nc.vector.tensor_scalar_add(v[:, 8, :], P[:, TL:TH], cbs)
                e9 = st("e9", GWN)
                nc.vector.tensor_add(e9[:], d1[:, TL:TH], m5[:, TL:TH])
                nc.vector.tensor_scalar_add(v[:, 9, :], e9[:], cbs)

            def gate_quad(g, eck0, tail=False):
                # 4 chains (eck0,c0),(eck0,c1),(eck0+1,c0),(eck0+1,c1) into
                # banks 0-3 of one psum slot, accumulated dck-MAJOR: the
                # cnn[dck=7]-dependent matmuls land ~5.7us of PE work after
                # a conv phase end (covers the trailing combine backlog),
                # and the deep chain pipeline avoids psum-WAR stalls.
                # tail=True (very last quad): chain 3 runs alone after the
                # other three so only its short split epilog trails the
                # kernel's last matmul.
                chains = [(eck0, 0), (eck0, 1), (eck0 + 1, 0), (eck0 + 1, 1)]

                def epilog(pst, q, eck, c, split=False):
                    gt = gout.tile([128, 480], mybir.dt.bfloat16, tag="gate")
                    ot = gout.tile([128, 480], mybir.dt.float32, tag="out")
                    chunks = ((0, 240), (240, 480)) if split else ((0, 480),)
                    for lo, hi in chunks:
                        nc.scalar.activation(gt[:, lo:hi],
                                             pst[:, q * 512 + lo:q * 512 + hi],
                                             AF.Sigmoid,
                                             bias=gb_sb[:, eck:eck + 1])
                        nc.vector.tensor_mul(ot[:, lo:hi],
                                             cnn_t[g][eck][:, c * 480 + lo:
                                                           c * 480 + hi],
                                             gt[:, lo:hi])
                        # the kernel's very last pieces issue from the (then
                        # idle) scalar queue so they don't queue behind the
                        # sync FIFO's earlier out-DMA issues
                        eng = nc.scalar if split else nc.sync
                        eng.dma_start(
                            outT[eck * 128:(eck + 1) * 128,
                                 g * GN + c * 480 + lo:g * GN + c * 480 + hi],
                            ot[:, lo:hi])

                if tail:
                    # chains 0-2 dck-major in ps2; chain 3 in its OWN
                    # rotated tile with matmuls emitted BEFORE the other
                    # epilogs (tile-granular WAR tracking would stall them
                    # on the sigmoids reading ps2), so only chain 3's short
                    # split epilog trails the kernel's final matmul.
                    ps2 = psum.tile([128, 2048], mybir.dt.float32, tag="cps")
                    for dck in range(NCH):
                        for q in range(3):
                            eck, c = chains[q]
                            nc.tensor.matmul(
                                ps2[:, q * 512:q * 512 + 480],
                                gwr_sb[eck][:, dck * 128:(dck + 1) * 128],
                                cnn_t[g][dck][:, c * 480:(c + 1) * 480],
                                start=(dck == 0),
                                stop=(dck == NCH - 1),
                            )
                    ps3 = psum.tile([128, 2048], mybir.dt.float32, tag="cps",
                                    name="tailq3")
                    eck3, c3 = chains[3]
                    for dck in range(NCH):
                        nc.tensor.matmul(
                            ps3[:, :480],
                            gwr_sb[eck3][:, dck * 128:(dck + 1) * 128],
                            cnn_t[g][dck][:, c3 * 480:(c3 + 1) * 480],
                            start=(dck == 0),
                            stop=(dck == NCH - 1),
                        )
                    for q in range(3):
                        epilog(ps2, q, *chains[q])
                    epilog(ps3, 0, eck3, c3, split=True)
                else:
                    ps2 = psum.tile([128, 2048], mybir.dt.float32, tag="cps")
                    for dck in range(NCH):
                        for q, (eck, c) in enumerate(chains):
                            nc.tensor.matmul(
                                ps2[:, q * 512:q * 512 + 480],
                                gwr_sb[eck][:, dck * 128:(dck + 1) * 128],
                                cnn_t[g][dck][:, c * 480:(c + 1) * 480],
                                start=(dck == 0),
                                stop=(dck == NCH - 1),
                            )
                    for q, (eck, c) in enumerate(chains):
                        epilog(ps2, q, eck, c)

            # phase g0 convs (cwr streams in behind; early units pad their
            # DMA-trickle idle with anti-idle matmuls)
            for dck in range(NCH):
                conv_unit(0, dck, pad_intra=(dck < 2),
                          pad_end=(4 if dck < 4 else 0))
            # xt g1 reloads during the g0 gate phase (WAR on the xt slots
            # releases at g0-conv end; the 25us gate window covers it)
            load_xt_half(1, 0)
            load_xt_half(1, 1)
            for eck0 in range(0, NCH, 2):
                gate_quad(0, eck0)
            for dck in range(NCH):
                conv_unit(1, dck)
            for eck0 in range(0, NCH, 2):
                gate_quad(1, eck0, tail=(eck0 == NCH - 2))
    nc.compile()
    return nc


def _prep_weights(conv_w, conv_b, gate_w, gate_b):
    W0, W1, W2 = [conv_w[:, :, k].astype(np.float64) for k in range(3)]
    g = []
    for b in SNODE:
        g.append(W2 if b is INF else W0 + b * W1 + b * b * W2)
    garr = np.stack(g)                                   # [6 storage, Do, Di]
    gv = garr.reshape(6, NCH, 128, NCH, 128)             # [s, dck, dd, ck, cc]
    cw_host = np.ascontiguousarray(gv.transpose(1, 4, 0, 3, 2)).reshape(
        NCH, 128, 6 * NCH * 128).astype(BF16)
    gwt = gate_w.T.reshape(NCH, 128, NCH, 128)           # [dck, dd, eck, ee]
    gw_host = np.ascontiguousarray(gwt.transpose(2, 1, 0, 3)).reshape(
        NCH, 128, NCH * 128).astype(BF16)
    cb_host = np.ascontiguousarray(conv_b.reshape(NCH, 128).T).astype(np.float32)
    gb_host = np.ascontiguousarray(gate_b.reshape(NCH, 128).T).astype(np.float32)
    return cw_host, gw_host, cb_host, gb_host


def _prep_core_x(x_shard):
    # x_shard [BC, T, D] -> xt [NG*2, 128, HCOLS]
    xw = x_shard.reshape(NWIN, W, D).astype(np.float64)
    xp = np.pad(xw, ((0, 0), (1, 1), (0, 0)))            # [192, 12, D]
    xt_host = np.empty((NG * 2, 128, HCOLS), BF16)
    for g in range(NG):
        ws = xp[g * GWN:(g + 1) * GWN]                   # [96, 12, D]
        xA = np.einsum('ji,wic->jwc', BA, ws[:, 0:6])    # [6, 96, D]
        xB = np.einsum('ji,wic->jwc', BA, ws[:, 4:10])
        xT = np.einsum('ji,wic->jwc', BT4, ws[:, 8:12])  # [4, 96, D]
        for s in range(6):
            j = LOGI[s]
            parts = [xA[j], xB[j]]
            if s in TROW:
                parts.append(xT[TROW[s]])
            S = np.concatenate(parts, axis=0)            # [WID[s], D]
            blk = S.T.reshape(NCH, 128, WID[s]).transpose(1, 0, 2)
            h = 0 if s < 3 else 1
            o = SOFF_X[s]
            xt_host[g * 2 + h, :, o:o + NCH * WID[s]] = \
                blk.reshape(128, NCH * WID[s]).astype(BF16)
    return xt_host


def _unshard_core(o):
    # o: [D, RC] cols ordered (g, t, w); group g == local batch row,
    # window w at in-window position t -> time w*10+t
    return np.ascontiguousarray(
        o.reshape(D, NG, W, GWN).transpose(1, 3, 2, 0).reshape(BC, T, D))


_NC_CACHE = None


def _prep_in_maps(x, conv_w, conv_b, gate_w, gate_b):
    cw_host, gw_host, cb_host, gb_host = _prep_weights(
        conv_w, conv_b, gate_w, gate_b)
    return [
        {"xt": _prep_core_x(x[BC * i:BC * (i + 1)]), "cwr": cw_host,
         "gwr": gw_host, "cb": cb_host, "gb": gb_host}
        for i in range(NCORES)
    ]


def kernel(x, conv_w, conv_b, gate_w, gate_b):
    global _NC_CACHE
    x = np.asarray(x, np.float32)
    conv_w = np.asarray(conv_w, np.float32)
    conv_b = np.asarray(conv_b, np.float32)
    gate_w = np.asarray(gate_w, np.float32)
    gate_b = np.asarray(gate_b, np.float32)

    in_maps = _prep_in_maps(x, conv_w, conv_b, gate_w, gate_b)
    if _NC_CACHE is None:
        _NC_CACHE = _build()
    res = run_bass_kernel_spmd(_NC_CACHE, in_maps,
                               core_ids=list(range(NCORES))).results

    out = np.empty((B, T, D), np.float32)
    for i in range(NCORES):
        out[BC * i:BC * (i + 1)] = _unshard_core(np.asarray(res[i]["outT"]))
    return out
